# revision 17
# baseline (speedup 1.0000x reference)
"""Trainium2 Bass kernel for a local-attention transformer block.

Data-parallel over tokens: 8 shards of 1024 tokens (+128-token halo).
Per core: transpose x to [d,tok]; QKV with large moving dims; attention
in transposed-score orientation (exp emits P^T directly, denominator via
an appended ones-column in the PV matmul, mask applied multiplicatively
after exp); out-proj + LN1 interleaved with attention; FFN with N=512
moving dims and gelu bias folded with n1_b@W1^T. Biases enter PSUM as
rank-1 matmuls or per-partition activation biases. Matmuls bf16 with
fp32 accumulation; softmax/layernorm kept fp32.
"""

import numpy as np
import ml_dtypes

# ---- problem constants (hardcoded per contract) ----
B, S, D = 2, 4096, 768
NH, HD = 12, 64
DFF = 4 * D            # 3072
DQK = 2 * D            # 1536
WIN = 128
EPS = 1e-5
T = 128
NB = 8                 # own 128-token blocks per core
NBH = NB + 1           # with halo block
NTOK = NB * T          # 1024
NTOKH = NBH * T        # 1152
ND = D // T            # 6
NF = DFF // T          # 24
N_CORES = 8

_CACHE = {}


def _build_nc(act="gelu"):
    import concourse.bacc as bacc
    import concourse.mybir as mybir
    from concourse import tile
    from concourse.masks import make_identity

    f32 = mybir.dt.float32
    bf16 = mybir.dt.bfloat16
    AF = mybir.ActivationFunctionType
    ALU = mybir.AluOpType

    nc = bacc.Bacc("TRN2", target_bir_lowering=False, debug=False,
                   num_devices=N_CORES)

    # ---- DRAM I/O ----
    xh_d = nc.dram_tensor("xh", [NTOKH, D], bf16, kind="ExternalInput").ap()
    wqk_d = nc.dram_tensor("wqkT", [D, DQK], bf16, kind="ExternalInput").ap()
    wv_d = nc.dram_tensor("wvT", [D, D], bf16, kind="ExternalInput").ap()
    wo_d = nc.dram_tensor("woT", [D, D], bf16, kind="ExternalInput").ap()
    w1_d = nc.dram_tensor("w1T", [D, DFF], bf16, kind="ExternalInput").ap()
    w2_d = nc.dram_tensor("w2T", [DFF, D], bf16, kind="ExternalInput").ap()
    qkb_d = nc.dram_tensor("qkb", [T, 2 * ND], f32, kind="ExternalInput").ap()
    gelub_d = nc.dram_tensor("gelub", [T, NF], f32, kind="ExternalInput").ap()
    obrep_d = nc.dram_tensor("obrep", [T, D], bf16, kind="ExternalInput").ap()
    b2rep_d = nc.dram_tensor("b2rep", [T, D], bf16, kind="ExternalInput").ap()
    g1rep_d = nc.dram_tensor("g1rep", [T, D], f32, kind="ExternalInput").ap()
    g2rep_d = nc.dram_tensor("g2rep", [T, D], f32, kind="ExternalInput").ap()
    n2brep_d = nc.dram_tensor("n2brep", [T, D], f32, kind="ExternalInput").ap()
    # masks in transposed [k, (h4, kb, q)] orientation, 0/1, tiled for 4 heads
    m01f_d = nc.dram_tensor("m01f", [T, 4 * 2 * T], bf16, kind="ExternalInput").ap()
    m01r_d = nc.dram_tensor("m01r", [T, 4 * 2 * T], bf16, kind="ExternalInput").ap()
    out_d = nc.dram_tensor("out", [NTOK, D], f32, kind="ExternalOutput").ap()

    with tile.TileContext(nc) as tc:
      with tc.tile_pool(name="persist", bufs=1) as persist, \
           tc.tile_pool(name="mid", bufs=1) as mid:
        ident = persist.tile([T, T], bf16, tag="ident")
        make_identity(nc, ident[:])
        ones_col = persist.tile([T, 1], bf16, tag="ones_col")
        nc.gpsimd.memset(ones_col[:], 1.0)
        # E0: row 0 all-ones selector for rank-1 bias adds (K=128)
        e0mat = persist.tile([T, T], bf16, tag="e0mat")
        nc.gpsimd.memset(e0mat[:], 0.0)
        nc.gpsimd.memset(e0mat[0:1, :], 1.0)
        eps_sb = persist.tile([T, 1], f32, tag="eps")
        nc.gpsimd.memset(eps_sb[:], EPS)
        qkb_sb = persist.tile([T, 2 * ND], f32, tag="qkb")
        gelub_sb = persist.tile([T, NF], f32, tag="gelub")
        obrep_sb = persist.tile([T, D], bf16, tag="obrep")
        b2rep_sb = persist.tile([T, D], bf16, tag="b2rep")
        g1rep_sb = persist.tile([T, D], f32, tag="g1rep")
        g2rep_sb = persist.tile([T, D], f32, tag="g2rep")
        n2brep_sb = persist.tile([T, D], f32, tag="n2brep")
        m01f_sb = persist.tile([T, 8 * T], bf16, tag="m01f")
        m01r_sb = persist.tile([T, 8 * T], bf16, tag="m01r")
        xh_sb = persist.tile([T, NBH, D], bf16, tag="xh")
        # xh leads the sync queue so transposes start ASAP
        xhr = xh_d.rearrange("(b p) d -> p b d", p=T)
        for c0 in range(0, NBH, 3):
            nc.sync.dma_start(xh_sb[:, c0:c0 + 3, :], xhr[:, c0:c0 + 3, :])
        nc.sync.dma_start(qkb_sb[:], qkb_d[:])
        nc.sync.dma_start(m01f_sb[:], m01f_d[:])
        nc.sync.dma_start(m01r_sb[:], m01r_d[:])
        nc.sync.dma_start(g1rep_sb[:], g1rep_d[:])
        nc.sync.dma_start(obrep_sb[:], obrep_d[:])
        nc.sync.dma_start(gelub_sb[:], gelub_d[:])
        nc.sync.dma_start(b2rep_sb[:], b2rep_d[:])
        nc.sync.dma_start(g2rep_sb[:], g2rep_d[:])
        nc.sync.dma_start(n2brep_sb[:], n2brep_d[:])
        # w1 preloaded early (tile only; DMA emitted after phase-A weights
        # so it doesn't delay them in the DMA queue)
        w1_sb = persist.tile([T, ND, DFF], bf16, tag="w1")

        x1_all = mid.tile([T, NB, D], bf16, tag="x1")
        x1T_all = mid.tile([T, ND, NTOK], bf16, tag="x1T")

        with tc.tile_pool(name="pqkv", bufs=1) as pqkv:
            qT = pqkv.tile([T, ND, NTOK], bf16, tag="qT")
            kT = pqkv.tile([T, ND, NTOKH], bf16, tag="kT")
            v_sb = pqkv.tile([T, NBH, D], bf16, tag="v")
            wo_sb = pqkv.tile([T, ND, D], bf16, tag="wo")
            yT_all = pqkv.tile([T, NB, ND, T], bf16, tag="yT")

            # ================= phase A: x^T, then Q/K/V ====================
            with tc.tile_pool(name="pa", bufs=1) as pa, \
                 tc.tile_pool(name="psA", bufs=2, space="PSUM") as psA:
                wqk_sb = pa.tile([T, ND, DQK], bf16, tag="wqk")
                nc.scalar.dma_start(wqk_sb[:],
                                    wqk_d.rearrange("(j p) n -> p j n", p=T))
                wv_sb = pa.tile([T, ND, D], bf16, tag="wv")
                nc.scalar.dma_start(wv_sb[:],
                                    wv_d.rearrange("(j p) n -> p j n", p=T))
                # later-phase weights queue behind the phase-A ones
                nc.scalar.dma_start(wo_sb[:],
                                    wo_d.rearrange("(j p) n -> p j n", p=T))
                nc.sync.dma_start(w1_sb[:],
                                  w1_d.rearrange("(j p) n -> p j n", p=T))
                xT_all = pa.tile([T, ND, NTOKH], bf16, tag="xT")

                for i in range(NBH):
                    ptr = psA.tile([T, ND, T], bf16, tag="xtr")
                    for j in range(ND):
                        nc.tensor.transpose(ptr[:, j, :],
                                            xh_sb[:, i, j * T:(j + 1) * T],
                                            ident[:])
                    nc.scalar.copy(xT_all[:, :, i * T:(i + 1) * T], ptr[:])

                # Q: own tokens only (2 groups of 512)
                for g in range(2):
                    tsl = slice(T + g * 512, T + (g + 1) * 512)
                    osl = slice(g * 512, (g + 1) * 512)
                    for cc in range(ND):
                        pq = psA.tile([T, 512], f32, tag="aq")
                        for j in range(ND):
                            nc.tensor.matmul(pq[:],
                                             wqk_sb[:, j, cc * T:(cc + 1) * T],
                                             xT_all[:, j, tsl],
                                             start=(j == 0), stop=(j == ND - 1))
                        nc.vector.tensor_scalar_add(qT[:, cc, osl], pq[:],
                                                    qkb_sb[:, cc:cc + 1])
                # K: halo'd tokens (3 groups of 384)
                for g in range(3):
                    tsl = slice(g * 384, (g + 1) * 384)
                    for cc in range(ND):
                        pk = psA.tile([T, 384], f32, tag="ak")
                        for j in range(ND):
                            nc.tensor.matmul(pk[:],
                                             wqk_sb[:, j, D + cc * T:D + (cc + 1) * T],
                                             xT_all[:, j, tsl],
                                             start=(j == 0), stop=(j == ND - 1))
                        nc.scalar.activation(kT[:, cc, tsl], pk[:], AF.Identity,
                                             bias=qkb_sb[:, ND + cc:ND + cc + 1])
                # V: [tok, ch] layout per block (bias folded into obrow)
                for i in range(NBH):
                    pv5 = psA.tile([T, 512], f32, tag="aq")
                    pv2 = psA.tile([T, 256], f32, tag="av2")
                    for j in range(ND):
                        nc.tensor.matmul(pv5[:], xT_all[:, j, i * T:(i + 1) * T],
                                         wv_sb[:, j, 0:512],
                                         start=(j == 0), stop=(j == ND - 1))
                    for j in range(ND):
                        nc.tensor.matmul(pv2[:], xT_all[:, j, i * T:(i + 1) * T],
                                         wv_sb[:, j, 512:768],
                                         start=(j == 0), stop=(j == ND - 1))
                    nc.vector.tensor_copy(v_sb[:, i, 0:512], pv5[:])
                    nc.scalar.copy(v_sb[:, i, 512:768], pv2[:])

            # ====== attention + B1 (out-proj + LN1 + x1^T), interleaved =====
            # PSUM budget (8 banks): st 2x2 + yp 1 + tr 1 + pz 2 = 8
            # QK matmuls grouped by operand partition offset: a po=0 -> 64
            # transition between consecutive matmuls into the same PSUM bank
            # faults on HW, so even heads (po=0) fill bank 0, odd heads bank 1.
            ORDER = (0, 2, 1, 3)
            RPOS = {0: 0, 2: 1, 1: 2, 3: 3}

            def emit_qk_chunk(t, c3, psS):
                ps_st = psS.tile([T, 4, 2, T], f32, tag="st", bufs=2)
                for ri, h4 in enumerate(ORDER):
                    h = c3 * 4 + h4
                    cc, po = h // 2, (h % 2) * HD
                    for kb in range(2):
                        nc.tensor.matmul(
                            ps_st[:, ri, kb, :],
                            kT[po:po + HD, cc, (t + kb) * T:(t + kb + 1) * T],
                            qT[po:po + HD, cc, t * T:(t + 1) * T],
                            start=True, stop=True)
                return ps_st

            def emit_attn(t, attn, psS, st_q):
                m01 = m01f_sb if t == 0 else m01r_sb
                y_blk = attn.tile([T, D], bf16, tag="yblk")
                for c3 in range(3):                      # 4 heads per chunk
                    ps_st = st_q.pop(0)
                    # queue the next chunk's QK matmuls behind this chunk's
                    # PV so the PE isn't idle while exp runs
                    if c3 < 2:
                        st_q.append(emit_qk_chunk(t, c3 + 1, psS))
                    P = attn.tile([T, 4, 2, T], bf16, tag="P")
                    nc.scalar.activation(P[:], ps_st[:], AF.Exp, scale=0.125)
                    nc.vector.tensor_tensor(P[:], P[:], m01[:], op=ALU.mult)
                    yp = psS.tile([T, 4, 80], f32, tag="yp", bufs=1)
                    nmm = 0
                    for h4 in range(4):
                        h = c3 * 4 + h4
                        ri = RPOS[h4]
                        for kb in range(2):
                            nc.tensor.matmul(
                                yp[:, h4, 0:HD], P[:, ri, kb, :],
                                v_sb[:, t + kb, h * HD:(h + 1) * HD],
                                start=(nmm == 0), stop=False,
                                skip_group_check=True)
                            nmm += 1
                            nc.tensor.matmul(
                                yp[:, h4, HD:HD + 1], P[:, ri, kb, :],
                                ones_col[:],
                                start=False, stop=(nmm == 7),
                                skip_group_check=True)
                            nmm += 1
                    rec = attn.tile([T, 4], f32, tag="rec")
                    nc.vector.reciprocal(rec[:], yp[:, :, HD])
                    for h4 in range(4):
                        h = c3 * 4 + h4
                        nc.vector.tensor_scalar_mul(
                            y_blk[:, h * HD:(h + 1) * HD], yp[:, h4, 0:HD],
                            rec[:, h4:h4 + 1])
                ptr = psS.tile([T, ND, T], bf16, tag="tr", bufs=1)
                for j in range(ND):
                    nc.tensor.transpose(ptr[:, j, :],
                                        y_blk[:, j * T:(j + 1) * T], ident[:])
                nc.vector.tensor_copy(yT_all[:, t, :, :], ptr[:])

            def emit_b1(t, wb, psB):
                pz = psB.tile([T, D], f32, tag="pz", bufs=1)
                for j in range(ND):
                    nc.tensor.matmul(pz[:, 0:512], yT_all[:, t, j, :],
                                     wo_sb[:, j, 0:512],
                                     start=(j == 0), stop=False)
                nc.tensor.matmul(pz[:, 0:512], e0mat[:], obrep_sb[:, 0:512],
                                 start=False, stop=True)
                for j in range(ND):
                    nc.tensor.matmul(pz[:, 512:768], yT_all[:, t, j, :],
                                     wo_sb[:, j, 512:768],
                                     start=(j == 0), stop=False)
                nc.tensor.matmul(pz[:, 512:768], e0mat[:],
                                 obrep_sb[:, 512:768],
                                 start=False, stop=True)
                x1pre = wb.tile([T, D], f32, tag="x1pre")
                s1a = wb.tile([T, 1], f32, tag="s1a")
                s1b = wb.tile([T, 1], f32, tag="s1b")
                nc.vector.scalar_tensor_tensor(
                    x1pre[:, 0:512], pz[:, 0:512], 1.0, xh_sb[:, t + 1, 0:512],
                    op0=ALU.mult, op1=ALU.add, accum_out=s1a[:])
                nc.vector.scalar_tensor_tensor(
                    x1pre[:, 512:768], pz[:, 512:768], 1.0,
                    xh_sb[:, t + 1, 512:768],
                    op0=ALU.mult, op1=ALU.add, accum_out=s1b[:])
                # LN1 (mean from accumulated sums)
                s1 = wb.tile([T, 1], f32, tag="s1")
                nc.vector.tensor_tensor(s1[:], s1a[:], s1b[:], op=ALU.add)
                nm = wb.tile([T, 1], f32, tag="nm")
                nc.scalar.mul(nm[:], s1[:], -1.0 / D)
                xc = wb.tile([T, D], f32, tag="xc")
                nc.vector.tensor_scalar_add(xc[:], x1pre[:], nm[:])
                sq = wb.tile([T, D], f32, tag="sq")
                vs = wb.tile([T, 1], f32, tag="vs")
                nc.vector.scalar_tensor_tensor(sq[:], xc[:], 1.0, xc[:],
                                               op0=ALU.mult, op1=ALU.mult,
                                               accum_out=vs[:])
                std = wb.tile([T, 1], f32, tag="std")
                nc.scalar.activation(std[:], vs[:], AF.Sqrt, bias=eps_sb[:],
                                     scale=1.0 / D)
                rstd = wb.tile([T, 1], f32, tag="rstd")
                nc.vector.reciprocal(rstd[:], std[:])
                nc.vector.scalar_tensor_tensor(x1_all[:, t, :], xc[:], rstd[:],
                                               g1rep_sb[:], op0=ALU.mult,
                                               op1=ALU.mult)
                ptr = psB.tile([T, ND, T], bf16, tag="tr", bufs=1)
                for j in range(ND):
                    nc.tensor.transpose(ptr[:, j, :],
                                        x1_all[:, t, j * T:(j + 1) * T],
                                        ident[:])
                nc.scalar.copy(x1T_all[:, :, t * T:(t + 1) * T], ptr[:])

            with tc.tile_pool(name="attn", bufs=3) as attn, \
                 tc.tile_pool(name="wb1w", bufs=2) as wb, \
                 tc.tile_pool(name="psS", bufs=1, space="PSUM") as psS:
                st_q = [emit_qk_chunk(0, 0, psS)]
                emit_attn(0, attn, psS, st_q)
                for t in range(1, NB):
                    st_q.append(emit_qk_chunk(t, 0, psS))
                    emit_b1(t - 1, wb, psS)
                    emit_attn(t, attn, psS, st_q)
                emit_b1(NB - 1, wb, psS)

        # ============= phase B2/B3: FFN + LN2, stage-major ==============
        with tc.tile_pool(name="wB2", bufs=1) as wB2, \
             tc.tile_pool(name="hbuf", bufs=1) as hbuf, \
             tc.tile_pool(name="workB", bufs=2) as workB, \
             tc.tile_pool(name="psF", bufs=2, space="PSUM") as psF:
            w2_sb = wB2.tile([T, NF, D], bf16, tag="w2")
            w2r = w2_d.rearrange("(j p) n -> p j n", p=T)
            nc.sync.dma_start(w2_sb[:, 0:12, :], w2r[:, 0:12, :])
            nc.sync.dma_start(w2_sb[:, 12:24, :], w2r[:, 12:24, :])

            def emit_b2(g, h_g):
                for fi in range(NF):
                    ph = psF.tile([T, 512], f32, tag="ph")
                    for j in range(ND):
                        nc.tensor.matmul(
                            ph[:], w1_sb[:, j, fi * T:(fi + 1) * T],
                            x1T_all[:, j, g * 512:(g + 1) * 512],
                            start=(j == 0), stop=(j == ND - 1))
                    act_fn = AF.Gelu if act == "gelu" else AF.Identity
                    nc.scalar.activation(h_g[:, fi, :], ph[:], act_fn,
                                         bias=gelub_sb[:, fi:fi + 1])

            def emit_b3(t, h_g):
                px5 = psF.tile([T, 512], f32, tag="px5")
                px2 = psF.tile([T, 256], f32, tag="px2")
                tin = (t % 4) * T
                for fi in range(NF):
                    nc.tensor.matmul(px5[:], h_g[:, fi, tin:tin + T],
                                     w2_sb[:, fi, 0:512],
                                     start=(fi == 0), stop=False)
                nc.tensor.matmul(px5[:], e0mat[:], b2rep_sb[:, 0:512],
                                 start=False, stop=True)
                for fi in range(NF):
                    nc.tensor.matmul(px2[:], h_g[:, fi, tin:tin + T],
                                     w2_sb[:, fi, 512:768],
                                     start=(fi == 0), stop=False)
                nc.tensor.matmul(px2[:], e0mat[:], b2rep_sb[:, 512:768],
                                 start=False, stop=True)
                x2pre = workB.tile([T, D], f32, tag="x2pre")
                s1a = workB.tile([T, 1], f32, tag="s1a")
                s1b = workB.tile([T, 1], f32, tag="s1b")
                nc.vector.scalar_tensor_tensor(
                    x2pre[:, 0:512], px5[:], 1.0, x1_all[:, t, 0:512],
                    op0=ALU.mult, op1=ALU.add, accum_out=s1a[:])
                nc.vector.scalar_tensor_tensor(
                    x2pre[:, 512:768], px2[:], 1.0, x1_all[:, t, 512:768],
                    op0=ALU.mult, op1=ALU.add, accum_out=s1b[:])
                s1 = workB.tile([T, 1], f32, tag="s1")
                nc.vector.tensor_tensor(s1[:], s1a[:], s1b[:], op=ALU.add)
                nm = workB.tile([T, 1], f32, tag="nm")
                nc.scalar.mul(nm[:], s1[:], -1.0 / D)
                xc = workB.tile([T, D], f32, tag="xc")
                nc.vector.tensor_scalar_add(xc[:], x2pre[:], nm[:])
                sq = workB.tile([T, D], f32, tag="sq")
                vs = workB.tile([T, 1], f32, tag="vs")
                nc.vector.scalar_tensor_tensor(sq[:], xc[:], 1.0, xc[:],
                                               op0=ALU.mult, op1=ALU.mult,
                                               accum_out=vs[:])
                std = workB.tile([T, 1], f32, tag="std")
                nc.scalar.activation(std[:], vs[:], AF.Sqrt, bias=eps_sb[:],
                                     scale=1.0 / D)
                rstd = workB.tile([T, 1], f32, tag="rstd")
                nc.vector.reciprocal(rstd[:], std[:])
                xg = workB.tile([T, D], f32, tag="xg")
                nc.vector.scalar_tensor_tensor(xg[:], xc[:], rstd[:],
                                               g2rep_sb[:], op0=ALU.mult,
                                               op1=ALU.mult)
                ob = workB.tile([T, D], f32, tag="outb")
                nc.vector.tensor_tensor(ob[:], xg[:], n2brep_sb[:],
                                        op=ALU.add)
                nc.sync.dma_start(out_d[t * T:(t + 1) * T, :], ob[:])

            h_g0 = hbuf.tile([T, NF, 512], bf16, tag="h")
            emit_b2(0, h_g0)
            for t in range(4):
                emit_b3(t, h_g0)
            h_g1 = hbuf.tile([T, NF, 512], bf16, tag="h")
            emit_b2(1, h_g1)
            for t in range(4, NB):
                emit_b3(t, h_g1)

    nc.compile()
    return nc


def _get_nc(act="gelu"):
    if act not in _CACHE:
        _CACHE[act] = _build_nc(act)
    return _CACHE[act]


def make_in_maps(x, in_proj_w, in_proj_b, out_w, out_b, ff_w1, ff_b1,
                 ff_w2, ff_b2, n1_g, n1_b, n2_g, n2_b):
    bf = ml_dtypes.bfloat16
    f32 = np.float32
    x = np.asarray(x, f32).reshape(B, S, D)
    in_proj_w = np.asarray(in_proj_w, f32)
    in_proj_b = np.asarray(in_proj_b, f32)
    out_w = np.asarray(out_w, f32)
    ff_w1 = np.asarray(ff_w1, f32)
    ff_w2 = np.asarray(ff_w2, f32)
    n1_b = np.asarray(n1_b, f32)

    v_b = in_proj_b[DQK:]
    ob_eff = np.asarray(out_b, f32) + v_b @ out_w.T          # V-bias folded
    gelub_eff = np.asarray(ff_b1, f32) + n1_b @ ff_w1.T      # n1_b folded
    b2row_eff = np.asarray(ff_b2, f32) + n1_b                # n1_b residual

    shared = {
        "wqkT": np.ascontiguousarray(in_proj_w[:DQK].T).astype(bf),
        "wvT": np.ascontiguousarray(in_proj_w[DQK:].T).astype(bf),
        "woT": np.ascontiguousarray(out_w.T).astype(bf),
        "w1T": np.ascontiguousarray(ff_w1.T).astype(bf),
        "w2T": np.ascontiguousarray(ff_w2.T).astype(bf),
        "qkb": np.ascontiguousarray(
            in_proj_b[:DQK].reshape(2 * ND, T).T),
        "gelub": np.ascontiguousarray(gelub_eff.reshape(NF, T).T),
        "obrep": np.ascontiguousarray(
            np.broadcast_to(ob_eff[None, :], (T, D))).astype(bf),
        "b2rep": np.ascontiguousarray(
            np.broadcast_to(b2row_eff[None, :], (T, D))).astype(bf),
        "g1rep": np.ascontiguousarray(
            np.broadcast_to(np.asarray(n1_g, f32)[None, :], (T, D))),
        "g2rep": np.ascontiguousarray(
            np.broadcast_to(np.asarray(n2_g, f32)[None, :], (T, D))),
        "n2brep": np.ascontiguousarray(
            np.broadcast_to(np.asarray(n2_b, f32)[None, :], (T, D))),
    }

    # masks in [k, (h4, kb, q)] layout, 0/1 bf16, tiled over 4 heads
    k_i = np.arange(T, dtype=np.int64)[:, None]
    q_i = np.arange(T, dtype=np.int64)[None, :]
    m_kb0 = (k_i > q_i).astype(f32)         # previous key block
    m_kb1 = (k_i <= q_i).astype(f32)        # current key block (causal)
    rest = np.concatenate([m_kb0, m_kb1], axis=1)          # [T, 2T]
    first = np.concatenate([np.zeros((T, T), f32), m_kb1], axis=1)
    m01r = np.ascontiguousarray(np.tile(rest, (1, 4))).astype(bf)
    m01f_bs = np.ascontiguousarray(np.tile(first, (1, 4))).astype(bf)

    in_maps = []
    for c in range(N_CORES):
        b, i0 = divmod(c * NTOK, S)
        halo = (np.zeros((T, D), f32) if i0 == 0 else x[b, i0 - T:i0])
        xh = np.ascontiguousarray(
            np.concatenate([halo, x[b, i0:i0 + NTOK]], axis=0)).astype(bf)
        m = dict(shared)
        m["xh"] = xh
        m["m01f"] = m01f_bs if i0 == 0 else m01r
        m["m01r"] = m01r
        in_maps.append(m)
    return in_maps


def kernel(**inputs):
    from concourse.bass_utils import run_bass_kernel_spmd
    nc = _get_nc()
    in_maps = make_in_maps(**inputs)
    res = run_bass_kernel_spmd(nc, in_maps, core_ids=list(range(N_CORES)))
    outs = [res.results[c]["out"] for c in range(N_CORES)]
    return np.concatenate(outs, axis=0).reshape(B, S, D).astype(np.float32)


# revision 18
# speedup vs baseline: 1.0240x; 1.0240x over previous
"""Trainium2 Bass kernel for a local-attention transformer block.

Data-parallel over tokens: 8 shards of 1024 tokens (+128-token halo).
Per core: transpose x to [d,tok]; QKV with large moving dims; attention
in transposed-score orientation (exp emits P^T directly, denominator via
an appended ones-column in the PV matmul, mask applied multiplicatively
after exp); out-proj + LN1 interleaved with attention; FFN with N=512
moving dims and gelu bias folded with n1_b@W1^T. Biases enter PSUM as
rank-1 matmuls or per-partition activation biases. Matmuls bf16 with
fp32 accumulation; softmax/layernorm kept fp32.
"""

import numpy as np
import ml_dtypes

# ---- problem constants (hardcoded per contract) ----
B, S, D = 2, 4096, 768
NH, HD = 12, 64
DFF = 4 * D            # 3072
DQK = 2 * D            # 1536
WIN = 128
EPS = 1e-5
T = 128
NB = 8                 # own 128-token blocks per core
NBH = NB + 1           # with halo block
NTOK = NB * T          # 1024
NTOKH = NBH * T        # 1152
ND = D // T            # 6
NF = DFF // T          # 24
N_CORES = 8

_CACHE = {}


def _build_nc(act="gelu"):
    import concourse.bacc as bacc
    import concourse.mybir as mybir
    from concourse import tile
    from concourse.masks import make_identity

    f32 = mybir.dt.float32
    bf16 = mybir.dt.bfloat16
    AF = mybir.ActivationFunctionType
    ALU = mybir.AluOpType

    nc = bacc.Bacc("TRN2", target_bir_lowering=False, debug=False,
                   num_devices=N_CORES)

    # ---- DRAM I/O ----
    xh_d = nc.dram_tensor("xh", [NTOKH, D], bf16, kind="ExternalInput").ap()
    wqk_d = nc.dram_tensor("wqkT", [D, DQK], bf16, kind="ExternalInput").ap()
    wv_d = nc.dram_tensor("wvT", [D, D], bf16, kind="ExternalInput").ap()
    wo_d = nc.dram_tensor("woT", [D, D], bf16, kind="ExternalInput").ap()
    w1_d = nc.dram_tensor("w1T", [D, DFF], bf16, kind="ExternalInput").ap()
    w2_d = nc.dram_tensor("w2T", [DFF, D], bf16, kind="ExternalInput").ap()
    qkb_d = nc.dram_tensor("qkb", [T, 2 * ND], f32, kind="ExternalInput").ap()
    gelub_d = nc.dram_tensor("gelub", [T, NF], f32, kind="ExternalInput").ap()
    obrep_d = nc.dram_tensor("obrep", [T, D], bf16, kind="ExternalInput").ap()
    b2rep_d = nc.dram_tensor("b2rep", [T, D], bf16, kind="ExternalInput").ap()
    g1rep_d = nc.dram_tensor("g1rep", [T, D], f32, kind="ExternalInput").ap()
    g2rep_d = nc.dram_tensor("g2rep", [T, D], f32, kind="ExternalInput").ap()
    n2brep_d = nc.dram_tensor("n2brep", [T, D], f32, kind="ExternalInput").ap()
    # masks in transposed [k, (h4, kb, q)] orientation, 0/1, tiled for 4 heads
    m01f_d = nc.dram_tensor("m01f", [T, 4 * 2 * T], bf16, kind="ExternalInput").ap()
    m01r_d = nc.dram_tensor("m01r", [T, 4 * 2 * T], bf16, kind="ExternalInput").ap()
    out_d = nc.dram_tensor("out", [NTOK, D], f32, kind="ExternalOutput").ap()

    with tile.TileContext(nc) as tc:
      with tc.tile_pool(name="persist", bufs=1) as persist, \
           tc.tile_pool(name="mid", bufs=1) as mid:
        ident = persist.tile([T, T], bf16, tag="ident")
        make_identity(nc, ident[:])
        ones_col = persist.tile([T, 1], bf16, tag="ones_col")
        nc.gpsimd.memset(ones_col[:], 1.0)
        # E0: row 0 all-ones selector for rank-1 bias adds (K=128)
        e0mat = persist.tile([T, T], bf16, tag="e0mat")
        nc.gpsimd.memset(e0mat[:], 0.0)
        nc.gpsimd.memset(e0mat[0:1, :], 1.0)
        eps_sb = persist.tile([T, 1], f32, tag="eps")
        nc.gpsimd.memset(eps_sb[:], EPS)
        qkb_sb = persist.tile([T, 2 * ND], f32, tag="qkb")
        gelub_sb = persist.tile([T, NF], f32, tag="gelub")
        obrep_sb = persist.tile([T, D], bf16, tag="obrep")
        b2rep_sb = persist.tile([T, D], bf16, tag="b2rep")
        g1rep_sb = persist.tile([T, D], f32, tag="g1rep")
        g2rep_sb = persist.tile([T, D], f32, tag="g2rep")
        n2brep_sb = persist.tile([T, D], f32, tag="n2brep")
        m01f_sb = persist.tile([T, 8 * T], bf16, tag="m01f")
        m01r_sb = persist.tile([T, 8 * T], bf16, tag="m01r")
        xh_sb = persist.tile([T, NBH, D], bf16, tag="xh")
        # xh leads the sync queue so transposes start ASAP
        xhr = xh_d.rearrange("(b p) d -> p b d", p=T)
        for c0 in range(0, NBH, 3):
            nc.sync.dma_start(xh_sb[:, c0:c0 + 3, :], xhr[:, c0:c0 + 3, :])
        nc.sync.dma_start(qkb_sb[:], qkb_d[:])
        nc.sync.dma_start(m01f_sb[:], m01f_d[:])
        nc.sync.dma_start(m01r_sb[:], m01r_d[:])
        nc.sync.dma_start(g1rep_sb[:], g1rep_d[:])
        nc.sync.dma_start(obrep_sb[:], obrep_d[:])
        nc.sync.dma_start(gelub_sb[:], gelub_d[:])
        nc.sync.dma_start(b2rep_sb[:], b2rep_d[:])
        nc.sync.dma_start(g2rep_sb[:], g2rep_d[:])
        nc.sync.dma_start(n2brep_sb[:], n2brep_d[:])
        # w1 preloaded early (tile only; DMA emitted after phase-A weights
        # so it doesn't delay them in the DMA queue)
        w1_sb = persist.tile([T, ND, DFF], bf16, tag="w1")

        x1_all = mid.tile([T, NB, D], bf16, tag="x1")
        x1T_all = mid.tile([T, ND, NTOK], bf16, tag="x1T")

        with tc.tile_pool(name="pqkv", bufs=1) as pqkv:
            qT = pqkv.tile([T, ND, NTOK], bf16, tag="qT")
            kT = pqkv.tile([T, ND, NTOKH], bf16, tag="kT")
            v_sb = pqkv.tile([T, NBH, D], bf16, tag="v")
            wo_sb = pqkv.tile([T, ND, D], bf16, tag="wo")
            yT_all = pqkv.tile([T, NB, ND, T], bf16, tag="yT")

            # ================= phase A: x^T, then Q/K/V ====================
            with tc.tile_pool(name="pa", bufs=1) as pa, \
                 tc.tile_pool(name="psA", bufs=2, space="PSUM") as psA:
                wqk_sb = pa.tile([T, ND, DQK], bf16, tag="wqk")
                nc.scalar.dma_start(wqk_sb[:],
                                    wqk_d.rearrange("(j p) n -> p j n", p=T))
                wv_sb = pa.tile([T, ND, D], bf16, tag="wv")
                nc.scalar.dma_start(wv_sb[:],
                                    wv_d.rearrange("(j p) n -> p j n", p=T))
                # later-phase weights queue behind the phase-A ones
                nc.scalar.dma_start(wo_sb[:],
                                    wo_d.rearrange("(j p) n -> p j n", p=T))
                nc.sync.dma_start(w1_sb[:],
                                  w1_d.rearrange("(j p) n -> p j n", p=T))
                xT_all = pa.tile([T, ND, NTOKH], bf16, tag="xT")

                for i in range(NBH):
                    ptr = psA.tile([T, ND, T], bf16, tag="xtr")
                    for j in range(ND):
                        nc.tensor.transpose(ptr[:, j, :],
                                            xh_sb[:, i, j * T:(j + 1) * T],
                                            ident[:])
                    nc.scalar.copy(xT_all[:, :, i * T:(i + 1) * T], ptr[:])

                # Q: own tokens only (2 groups of 512)
                for g in range(2):
                    tsl = slice(T + g * 512, T + (g + 1) * 512)
                    osl = slice(g * 512, (g + 1) * 512)
                    for cc in range(ND):
                        pq = psA.tile([T, 512], f32, tag="aq")
                        for j in range(ND):
                            nc.tensor.matmul(pq[:],
                                             wqk_sb[:, j, cc * T:(cc + 1) * T],
                                             xT_all[:, j, tsl],
                                             start=(j == 0), stop=(j == ND - 1))
                        nc.vector.tensor_scalar_add(qT[:, cc, osl], pq[:],
                                                    qkb_sb[:, cc:cc + 1])
                # K: halo'd tokens (3 groups of 384)
                for g in range(3):
                    tsl = slice(g * 384, (g + 1) * 384)
                    for cc in range(ND):
                        pk = psA.tile([T, 384], f32, tag="ak")
                        for j in range(ND):
                            nc.tensor.matmul(pk[:],
                                             wqk_sb[:, j, D + cc * T:D + (cc + 1) * T],
                                             xT_all[:, j, tsl],
                                             start=(j == 0), stop=(j == ND - 1))
                        nc.scalar.activation(kT[:, cc, tsl], pk[:], AF.Identity,
                                             bias=qkb_sb[:, ND + cc:ND + cc + 1])
                # V: [tok, ch] layout per block (bias folded into obrow)
                for i in range(NBH):
                    pv5 = psA.tile([T, 512], f32, tag="aq")
                    pv2 = psA.tile([T, 256], f32, tag="av2")
                    for j in range(ND):
                        nc.tensor.matmul(pv5[:], xT_all[:, j, i * T:(i + 1) * T],
                                         wv_sb[:, j, 0:512],
                                         start=(j == 0), stop=(j == ND - 1))
                    for j in range(ND):
                        nc.tensor.matmul(pv2[:], xT_all[:, j, i * T:(i + 1) * T],
                                         wv_sb[:, j, 512:768],
                                         start=(j == 0), stop=(j == ND - 1))
                    nc.vector.tensor_copy(v_sb[:, i, 0:512], pv5[:])
                    nc.scalar.copy(v_sb[:, i, 512:768], pv2[:])

            # ====== attention + B1 (out-proj + LN1 + x1^T), interleaved =====
            # PSUM budget (8 banks): st 2x2 + yp 1 + tr 1 + pz 2 = 8
            # QK matmuls grouped by operand partition offset: a po=0 -> 64
            # transition between consecutive matmuls into the same PSUM bank
            # faults on HW, so even heads (po=0) fill bank 0, odd heads bank 1.
            ORDER = (0, 2, 1, 3)
            RPOS = {0: 0, 2: 1, 1: 2, 3: 3}

            def emit_qk_chunk(t, c3, psS):
                ps_st = psS.tile([T, 4, 2, T], f32, tag="st", bufs=2)
                for ri, h4 in enumerate(ORDER):
                    h = c3 * 4 + h4
                    cc, po = h // 2, (h % 2) * HD
                    for kb in range(2):
                        nc.tensor.matmul(
                            ps_st[:, ri, kb, :],
                            kT[po:po + HD, cc, (t + kb) * T:(t + kb + 1) * T],
                            qT[po:po + HD, cc, t * T:(t + 1) * T],
                            start=True, stop=True)
                return ps_st

            def emit_attn(t, attn, psS, st_q, mid_cb=None):
                m01 = m01f_sb if t == 0 else m01r_sb
                y_blk = attn.tile([T, D], bf16, tag="yblk")
                for c3 in range(3):                      # 4 heads per chunk
                    ps_st = st_q.pop(0)
                    # queue the next chunk's QK matmuls behind this chunk's
                    # PV so the PE isn't idle while exp runs
                    if c3 < 2:
                        st_q.append(emit_qk_chunk(t, c3 + 1, psS))
                    P = attn.tile([T, 4, 2, T], bf16, tag="P")
                    nc.scalar.activation(P[:], ps_st[:], AF.Exp, scale=0.125)
                    nc.vector.tensor_tensor(P[:], P[:], m01[:], op=ALU.mult)
                    yp = psS.tile([T, 4, 80], f32, tag="yp", bufs=1)
                    nmm = 0
                    for h4 in range(4):
                        h = c3 * 4 + h4
                        ri = RPOS[h4]
                        for kb in range(2):
                            nc.tensor.matmul(
                                yp[:, h4, 0:HD], P[:, ri, kb, :],
                                v_sb[:, t + kb, h * HD:(h + 1) * HD],
                                start=(nmm == 0), stop=False,
                                skip_group_check=True)
                            nmm += 1
                            nc.tensor.matmul(
                                yp[:, h4, HD:HD + 1], P[:, ri, kb, :],
                                ones_col[:],
                                start=False, stop=(nmm == 7),
                                skip_group_check=True)
                            nmm += 1
                    rec = attn.tile([T, 4], f32, tag="rec")
                    nc.vector.reciprocal(rec[:], yp[:, :, HD])
                    for h4 in range(4):
                        h = c3 * 4 + h4
                        nc.vector.tensor_scalar_mul(
                            y_blk[:, h * HD:(h + 1) * HD], yp[:, h4, 0:HD],
                            rec[:, h4:h4 + 1])
                    if c3 == 0 and mid_cb is not None:
                        mid_cb()
                ptr = psS.tile([T, ND, T], bf16, tag="tr", bufs=1)
                for j in range(ND):
                    nc.tensor.transpose(ptr[:, j, :],
                                        y_blk[:, j * T:(j + 1) * T], ident[:])
                nc.vector.tensor_copy(yT_all[:, t, :, :], ptr[:])

            def emit_b1_mm(t, wb, psB):
                pz = psB.tile([T, D], f32, tag="pz", bufs=1)
                for j in range(ND):
                    nc.tensor.matmul(pz[:, 0:512], yT_all[:, t, j, :],
                                     wo_sb[:, j, 0:512],
                                     start=(j == 0), stop=False)
                nc.tensor.matmul(pz[:, 0:512], e0mat[:], obrep_sb[:, 0:512],
                                 start=False, stop=True)
                for j in range(ND):
                    nc.tensor.matmul(pz[:, 512:768], yT_all[:, t, j, :],
                                     wo_sb[:, j, 512:768],
                                     start=(j == 0), stop=False)
                nc.tensor.matmul(pz[:, 512:768], e0mat[:],
                                 obrep_sb[:, 512:768],
                                 start=False, stop=True)
                return pz

            def emit_b1_res(t, wb, pz):
                x1pre = wb.tile([T, D], f32, tag="x1pre")
                s1a = wb.tile([T, 1], f32, tag="s1a")
                s1b = wb.tile([T, 1], f32, tag="s1b")
                nc.vector.scalar_tensor_tensor(
                    x1pre[:, 0:512], pz[:, 0:512], 1.0, xh_sb[:, t + 1, 0:512],
                    op0=ALU.mult, op1=ALU.add, accum_out=s1a[:])
                nc.vector.scalar_tensor_tensor(
                    x1pre[:, 512:768], pz[:, 512:768], 1.0,
                    xh_sb[:, t + 1, 512:768],
                    op0=ALU.mult, op1=ALU.add, accum_out=s1b[:])
                return x1pre, s1a, s1b

            def emit_b1_ln(t, wb, psB, x1pre, s1a, s1b):
                s1 = wb.tile([T, 1], f32, tag="s1")
                nc.vector.tensor_tensor(s1[:], s1a[:], s1b[:], op=ALU.add)
                nm = wb.tile([T, 1], f32, tag="nm")
                nc.scalar.mul(nm[:], s1[:], -1.0 / D)
                xc = wb.tile([T, D], f32, tag="xc")
                nc.vector.tensor_scalar_add(xc[:], x1pre[:], nm[:])
                sq = wb.tile([T, D], f32, tag="sq")
                vs = wb.tile([T, 1], f32, tag="vs")
                nc.vector.scalar_tensor_tensor(sq[:], xc[:], 1.0, xc[:],
                                               op0=ALU.mult, op1=ALU.mult,
                                               accum_out=vs[:])
                std = wb.tile([T, 1], f32, tag="std")
                nc.scalar.activation(std[:], vs[:], AF.Sqrt, bias=eps_sb[:],
                                     scale=1.0 / D)
                rstd = wb.tile([T, 1], f32, tag="rstd")
                nc.vector.reciprocal(rstd[:], std[:])
                nc.vector.scalar_tensor_tensor(x1_all[:, t, :], xc[:], rstd[:],
                                               g1rep_sb[:], op0=ALU.mult,
                                               op1=ALU.mult)
                ptr = psB.tile([T, ND, T], bf16, tag="tr", bufs=1)
                for j in range(ND):
                    nc.tensor.transpose(ptr[:, j, :],
                                        x1_all[:, t, j * T:(j + 1) * T],
                                        ident[:])
                nc.scalar.copy(x1T_all[:, :, t * T:(t + 1) * T], ptr[:])

            with tc.tile_pool(name="attn", bufs=3) as attn, \
                 tc.tile_pool(name="wb1w", bufs=2) as wb, \
                 tc.tile_pool(name="psS", bufs=1, space="PSUM") as psS:
                st_q = [emit_qk_chunk(0, 0, psS)]
                emit_attn(0, attn, psS, st_q)
                for t in range(1, NB):
                    st_q.append(emit_qk_chunk(t, 0, psS))
                    pz = emit_b1_mm(t - 1, wb, psS)
                    res_out = []

                    def _mid(tt=t - 1, pzz=pz, ro=res_out):
                        ro.extend(emit_b1_res(tt, wb, pzz))

                    emit_attn(t, attn, psS, st_q, mid_cb=_mid)
                    emit_b1_ln(t - 1, wb, psS, *res_out)
                pz = emit_b1_mm(NB - 1, wb, psS)
                x1pre, s1a, s1b = emit_b1_res(NB - 1, wb, pz)
                emit_b1_ln(NB - 1, wb, psS, x1pre, s1a, s1b)

        # ============= phase B2/B3: FFN + LN2, stage-major ==============
        with tc.tile_pool(name="wB2", bufs=1) as wB2, \
             tc.tile_pool(name="hbuf", bufs=1) as hbuf, \
             tc.tile_pool(name="workB", bufs=2) as workB, \
             tc.tile_pool(name="psF", bufs=2, space="PSUM") as psF:
            w2_sb = wB2.tile([T, NF, D], bf16, tag="w2")
            w2r = w2_d.rearrange("(j p) n -> p j n", p=T)
            nc.sync.dma_start(w2_sb[:, 0:12, :], w2r[:, 0:12, :])
            nc.sync.dma_start(w2_sb[:, 12:24, :], w2r[:, 12:24, :])

            def emit_b2(g, h_g):
                for fi in range(NF):
                    ph = psF.tile([T, 512], f32, tag="ph")
                    for j in range(ND):
                        nc.tensor.matmul(
                            ph[:], w1_sb[:, j, fi * T:(fi + 1) * T],
                            x1T_all[:, j, g * 512:(g + 1) * 512],
                            start=(j == 0), stop=(j == ND - 1))
                    act_fn = AF.Gelu if act == "gelu" else AF.Identity
                    nc.scalar.activation(h_g[:, fi, :], ph[:], act_fn,
                                         bias=gelub_sb[:, fi:fi + 1])

            def emit_b3(t, h_g):
                px5 = psF.tile([T, 512], f32, tag="px5")
                px2 = psF.tile([T, 256], f32, tag="px2")
                tin = (t % 4) * T
                for fi in range(NF):
                    nc.tensor.matmul(px5[:], h_g[:, fi, tin:tin + T],
                                     w2_sb[:, fi, 0:512],
                                     start=(fi == 0), stop=False)
                nc.tensor.matmul(px5[:], e0mat[:], b2rep_sb[:, 0:512],
                                 start=False, stop=True)
                for fi in range(NF):
                    nc.tensor.matmul(px2[:], h_g[:, fi, tin:tin + T],
                                     w2_sb[:, fi, 512:768],
                                     start=(fi == 0), stop=False)
                nc.tensor.matmul(px2[:], e0mat[:], b2rep_sb[:, 512:768],
                                 start=False, stop=True)
                x2pre = workB.tile([T, D], f32, tag="x2pre")
                s1a = workB.tile([T, 1], f32, tag="s1a")
                s1b = workB.tile([T, 1], f32, tag="s1b")
                nc.vector.scalar_tensor_tensor(
                    x2pre[:, 0:512], px5[:], 1.0, x1_all[:, t, 0:512],
                    op0=ALU.mult, op1=ALU.add, accum_out=s1a[:])
                nc.vector.scalar_tensor_tensor(
                    x2pre[:, 512:768], px2[:], 1.0, x1_all[:, t, 512:768],
                    op0=ALU.mult, op1=ALU.add, accum_out=s1b[:])
                s1 = workB.tile([T, 1], f32, tag="s1")
                nc.vector.tensor_tensor(s1[:], s1a[:], s1b[:], op=ALU.add)
                nm = workB.tile([T, 1], f32, tag="nm")
                nc.scalar.mul(nm[:], s1[:], -1.0 / D)
                xc = workB.tile([T, D], f32, tag="xc")
                nc.vector.tensor_scalar_add(xc[:], x2pre[:], nm[:])
                sq = workB.tile([T, D], f32, tag="sq")
                vs = workB.tile([T, 1], f32, tag="vs")
                nc.vector.scalar_tensor_tensor(sq[:], xc[:], 1.0, xc[:],
                                               op0=ALU.mult, op1=ALU.mult,
                                               accum_out=vs[:])
                std = workB.tile([T, 1], f32, tag="std")
                nc.scalar.activation(std[:], vs[:], AF.Sqrt, bias=eps_sb[:],
                                     scale=1.0 / D)
                rstd = workB.tile([T, 1], f32, tag="rstd")
                nc.vector.reciprocal(rstd[:], std[:])
                xg = workB.tile([T, D], f32, tag="xg")
                nc.vector.scalar_tensor_tensor(xg[:], xc[:], rstd[:],
                                               g2rep_sb[:], op0=ALU.mult,
                                               op1=ALU.mult)
                ob = workB.tile([T, D], f32, tag="outb")
                nc.vector.tensor_tensor(ob[:], xg[:], n2brep_sb[:],
                                        op=ALU.add)
                nc.sync.dma_start(out_d[t * T:(t + 1) * T, :], ob[:])

            h_g0 = hbuf.tile([T, NF, 512], bf16, tag="h")
            emit_b2(0, h_g0)
            for t in range(4):
                emit_b3(t, h_g0)
            h_g1 = hbuf.tile([T, NF, 512], bf16, tag="h")
            emit_b2(1, h_g1)
            for t in range(4, NB):
                emit_b3(t, h_g1)

    nc.compile()
    return nc


def _get_nc(act="gelu"):
    if act not in _CACHE:
        _CACHE[act] = _build_nc(act)
    return _CACHE[act]


def make_in_maps(x, in_proj_w, in_proj_b, out_w, out_b, ff_w1, ff_b1,
                 ff_w2, ff_b2, n1_g, n1_b, n2_g, n2_b):
    bf = ml_dtypes.bfloat16
    f32 = np.float32
    x = np.asarray(x, f32).reshape(B, S, D)
    in_proj_w = np.asarray(in_proj_w, f32)
    in_proj_b = np.asarray(in_proj_b, f32)
    out_w = np.asarray(out_w, f32)
    ff_w1 = np.asarray(ff_w1, f32)
    ff_w2 = np.asarray(ff_w2, f32)
    n1_b = np.asarray(n1_b, f32)

    v_b = in_proj_b[DQK:]
    ob_eff = np.asarray(out_b, f32) + v_b @ out_w.T          # V-bias folded
    gelub_eff = np.asarray(ff_b1, f32) + n1_b @ ff_w1.T      # n1_b folded
    b2row_eff = np.asarray(ff_b2, f32) + n1_b                # n1_b residual

    shared = {
        "wqkT": np.ascontiguousarray(in_proj_w[:DQK].T).astype(bf),
        "wvT": np.ascontiguousarray(in_proj_w[DQK:].T).astype(bf),
        "woT": np.ascontiguousarray(out_w.T).astype(bf),
        "w1T": np.ascontiguousarray(ff_w1.T).astype(bf),
        "w2T": np.ascontiguousarray(ff_w2.T).astype(bf),
        "qkb": np.ascontiguousarray(
            in_proj_b[:DQK].reshape(2 * ND, T).T),
        "gelub": np.ascontiguousarray(gelub_eff.reshape(NF, T).T),
        "obrep": np.ascontiguousarray(
            np.broadcast_to(ob_eff[None, :], (T, D))).astype(bf),
        "b2rep": np.ascontiguousarray(
            np.broadcast_to(b2row_eff[None, :], (T, D))).astype(bf),
        "g1rep": np.ascontiguousarray(
            np.broadcast_to(np.asarray(n1_g, f32)[None, :], (T, D))),
        "g2rep": np.ascontiguousarray(
            np.broadcast_to(np.asarray(n2_g, f32)[None, :], (T, D))),
        "n2brep": np.ascontiguousarray(
            np.broadcast_to(np.asarray(n2_b, f32)[None, :], (T, D))),
    }

    # masks in [k, (h4, kb, q)] layout, 0/1 bf16, tiled over 4 heads
    k_i = np.arange(T, dtype=np.int64)[:, None]
    q_i = np.arange(T, dtype=np.int64)[None, :]
    m_kb0 = (k_i > q_i).astype(f32)         # previous key block
    m_kb1 = (k_i <= q_i).astype(f32)        # current key block (causal)
    rest = np.concatenate([m_kb0, m_kb1], axis=1)          # [T, 2T]
    first = np.concatenate([np.zeros((T, T), f32), m_kb1], axis=1)
    m01r = np.ascontiguousarray(np.tile(rest, (1, 4))).astype(bf)
    m01f_bs = np.ascontiguousarray(np.tile(first, (1, 4))).astype(bf)

    in_maps = []
    for c in range(N_CORES):
        b, i0 = divmod(c * NTOK, S)
        halo = (np.zeros((T, D), f32) if i0 == 0 else x[b, i0 - T:i0])
        xh = np.ascontiguousarray(
            np.concatenate([halo, x[b, i0:i0 + NTOK]], axis=0)).astype(bf)
        m = dict(shared)
        m["xh"] = xh
        m["m01f"] = m01f_bs if i0 == 0 else m01r
        m["m01r"] = m01r
        in_maps.append(m)
    return in_maps


def kernel(**inputs):
    from concourse.bass_utils import run_bass_kernel_spmd
    nc = _get_nc()
    in_maps = make_in_maps(**inputs)
    res = run_bass_kernel_spmd(nc, in_maps, core_ids=list(range(N_CORES)))
    outs = [res.results[c]["out"] for c in range(N_CORES)]
    return np.concatenate(outs, axis=0).reshape(B, S, D).astype(np.float32)


# revision 19
# speedup vs baseline: 1.0621x; 1.0372x over previous
"""Trainium2 Bass kernel for a local-attention transformer block.

Data-parallel over tokens: 8 shards of 1024 tokens (+128-token halo).
Per core: transpose x to [d,tok]; QKV with large moving dims; attention
in transposed-score orientation (exp emits P^T directly, denominator via
an appended ones-column in the PV matmul, mask applied multiplicatively
after exp); out-proj + LN1 interleaved with attention; FFN with N=512
moving dims and gelu bias folded with n1_b@W1^T. Biases enter PSUM as
rank-1 matmuls or per-partition activation biases. Matmuls bf16 with
fp32 accumulation; softmax/layernorm kept fp32.
"""

import numpy as np
import ml_dtypes

# ---- problem constants (hardcoded per contract) ----
B, S, D = 2, 4096, 768
NH, HD = 12, 64
DFF = 4 * D            # 3072
DQK = 2 * D            # 1536
WIN = 128
EPS = 1e-5
T = 128
NB = 8                 # own 128-token blocks per core
NBH = NB + 1           # with halo block
NTOK = NB * T          # 1024
NTOKH = NBH * T        # 1152
ND = D // T            # 6
NF = DFF // T          # 24
N_CORES = 8

_CACHE = {}


def _build_nc(act="gelu"):
    import concourse.bacc as bacc
    import concourse.mybir as mybir
    from concourse import tile
    from concourse.masks import make_identity

    f32 = mybir.dt.float32
    bf16 = mybir.dt.bfloat16
    AF = mybir.ActivationFunctionType
    ALU = mybir.AluOpType

    nc = bacc.Bacc("TRN2", target_bir_lowering=False, debug=False,
                   num_devices=N_CORES)

    # ---- DRAM I/O ----
    xh_d = nc.dram_tensor("xh", [NTOKH, D], bf16, kind="ExternalInput").ap()
    wqk_d = nc.dram_tensor("wqkT", [D, DQK], bf16, kind="ExternalInput").ap()
    wv_d = nc.dram_tensor("wvT", [D, D], bf16, kind="ExternalInput").ap()
    wo_d = nc.dram_tensor("woT", [D, D], bf16, kind="ExternalInput").ap()
    w1_d = nc.dram_tensor("w1T", [D, DFF], bf16, kind="ExternalInput").ap()
    w2_d = nc.dram_tensor("w2T", [DFF, D], bf16, kind="ExternalInput").ap()
    qkb_d = nc.dram_tensor("qkb", [T, 2 * ND], f32, kind="ExternalInput").ap()
    gelub_d = nc.dram_tensor("gelub", [T, NF], f32, kind="ExternalInput").ap()
    obrep_d = nc.dram_tensor("obrep", [T, D], bf16, kind="ExternalInput").ap()
    b2rep_d = nc.dram_tensor("b2rep", [T, D], bf16, kind="ExternalInput").ap()
    g1rep_d = nc.dram_tensor("g1rep", [T, D], f32, kind="ExternalInput").ap()
    g2rep_d = nc.dram_tensor("g2rep", [T, D], f32, kind="ExternalInput").ap()
    n2brep_d = nc.dram_tensor("n2brep", [T, D], f32, kind="ExternalInput").ap()
    # masks in transposed [k, (h4, kb, q)] orientation, 0/1, tiled for 4 heads
    m01f_d = nc.dram_tensor("m01f", [T, 4 * 2 * T], bf16, kind="ExternalInput").ap()
    m01r_d = nc.dram_tensor("m01r", [T, 4 * 2 * T], bf16, kind="ExternalInput").ap()
    out_d = nc.dram_tensor("out", [NTOK, D], f32, kind="ExternalOutput").ap()

    with tile.TileContext(nc) as tc:
      with tc.tile_pool(name="persist", bufs=1) as persist, \
           tc.tile_pool(name="mid", bufs=1) as mid:
        ident = persist.tile([T, T], bf16, tag="ident")
        make_identity(nc, ident[:])
        ones_col = persist.tile([T, 1], bf16, tag="ones_col")
        nc.gpsimd.memset(ones_col[:], 1.0)
        # E0: row 0 all-ones selector for rank-1 bias adds (K=128)
        e0mat = persist.tile([T, T], bf16, tag="e0mat")
        nc.gpsimd.memset(e0mat[:], 0.0)
        nc.gpsimd.memset(e0mat[0:1, :], 1.0)
        eps_sb = persist.tile([T, 1], f32, tag="eps")
        nc.gpsimd.memset(eps_sb[:], EPS)
        qkb_sb = persist.tile([T, 2 * ND], f32, tag="qkb")
        nc.sync.dma_start(qkb_sb[:], qkb_d[:])
        gelub_sb = persist.tile([T, NF], f32, tag="gelub")
        nc.sync.dma_start(gelub_sb[:], gelub_d[:])
        obrep_sb = persist.tile([T, D], bf16, tag="obrep")
        nc.sync.dma_start(obrep_sb[:], obrep_d[:])
        b2rep_sb = persist.tile([T, D], bf16, tag="b2rep")
        nc.sync.dma_start(b2rep_sb[:], b2rep_d[:])
        g1rep_sb = persist.tile([T, D], f32, tag="g1rep")
        nc.sync.dma_start(g1rep_sb[:], g1rep_d[:])
        g2rep_sb = persist.tile([T, D], f32, tag="g2rep")
        nc.sync.dma_start(g2rep_sb[:], g2rep_d[:])
        n2brep_sb = persist.tile([T, D], f32, tag="n2brep")
        nc.sync.dma_start(n2brep_sb[:], n2brep_d[:])
        m01f_sb = persist.tile([T, 8 * T], bf16, tag="m01f")
        nc.sync.dma_start(m01f_sb[:], m01f_d[:])
        m01r_sb = persist.tile([T, 8 * T], bf16, tag="m01r")
        nc.sync.dma_start(m01r_sb[:], m01r_d[:])
        xh_sb = persist.tile([T, NBH, D], bf16, tag="xh")
        nc.sync.dma_start(xh_sb[:], xh_d.rearrange("(b p) d -> p b d", p=T))
        # w1 preloaded early (tile only; DMA emitted after phase-A weights
        # so it doesn't delay them in the DMA queue)
        w1_sb = persist.tile([T, ND, DFF], bf16, tag="w1")

        x1_all = mid.tile([T, NB, D], bf16, tag="x1")
        x1T_all = mid.tile([T, ND, NTOK], bf16, tag="x1T")

        with tc.tile_pool(name="pqkv", bufs=1) as pqkv:
            qT = pqkv.tile([T, ND, NTOK], bf16, tag="qT")
            kT = pqkv.tile([T, ND, NTOKH], bf16, tag="kT")
            v_sb = pqkv.tile([T, NBH, D], bf16, tag="v")
            wo_sb = pqkv.tile([T, ND, D], bf16, tag="wo")
            yT_all = pqkv.tile([T, NB, ND, T], bf16, tag="yT")

            # ================= phase A: x^T, then Q/K/V ====================
            with tc.tile_pool(name="pa", bufs=1) as pa, \
                 tc.tile_pool(name="psA", bufs=2, space="PSUM") as psA:
                wqk_sb = pa.tile([T, ND, DQK], bf16, tag="wqk")
                nc.sync.dma_start(wqk_sb[:],
                                  wqk_d.rearrange("(j p) n -> p j n", p=T))
                wv_sb = pa.tile([T, ND, D], bf16, tag="wv")
                nc.sync.dma_start(wv_sb[:],
                                  wv_d.rearrange("(j p) n -> p j n", p=T))
                # later-phase weights queue behind the phase-A ones
                nc.sync.dma_start(wo_sb[:],
                                  wo_d.rearrange("(j p) n -> p j n", p=T))
                nc.sync.dma_start(w1_sb[:],
                                  w1_d.rearrange("(j p) n -> p j n", p=T))
                xT_all = pa.tile([T, ND, NTOKH], bf16, tag="xT")

                for i in range(NBH):
                    ptr = psA.tile([T, ND, T], bf16, tag="xtr")
                    for j in range(ND):
                        nc.tensor.transpose(ptr[:, j, :],
                                            xh_sb[:, i, j * T:(j + 1) * T],
                                            ident[:])
                    nc.scalar.copy(xT_all[:, :, i * T:(i + 1) * T], ptr[:])

                # Q: own tokens only (2 groups of 512)
                for g in range(2):
                    tsl = slice(T + g * 512, T + (g + 1) * 512)
                    osl = slice(g * 512, (g + 1) * 512)
                    for cc in range(ND):
                        pq = psA.tile([T, 512], f32, tag="aq")
                        for j in range(ND):
                            nc.tensor.matmul(pq[:],
                                             wqk_sb[:, j, cc * T:(cc + 1) * T],
                                             xT_all[:, j, tsl],
                                             start=(j == 0), stop=(j == ND - 1))
                        nc.vector.tensor_scalar_add(qT[:, cc, osl], pq[:],
                                                    qkb_sb[:, cc:cc + 1])
                # K: halo'd tokens (3 groups of 384)
                for g in range(3):
                    tsl = slice(g * 384, (g + 1) * 384)
                    for cc in range(ND):
                        pk = psA.tile([T, 384], f32, tag="ak")
                        for j in range(ND):
                            nc.tensor.matmul(pk[:],
                                             wqk_sb[:, j, D + cc * T:D + (cc + 1) * T],
                                             xT_all[:, j, tsl],
                                             start=(j == 0), stop=(j == ND - 1))
                        nc.scalar.activation(kT[:, cc, tsl], pk[:], AF.Identity,
                                             bias=qkb_sb[:, ND + cc:ND + cc + 1])
                # V: [tok, ch] layout per block (bias folded into obrow)
                for i in range(NBH):
                    pv5 = psA.tile([T, 512], f32, tag="aq")
                    pv2 = psA.tile([T, 256], f32, tag="av2")
                    for j in range(ND):
                        nc.tensor.matmul(pv5[:], xT_all[:, j, i * T:(i + 1) * T],
                                         wv_sb[:, j, 0:512],
                                         start=(j == 0), stop=(j == ND - 1))
                    for j in range(ND):
                        nc.tensor.matmul(pv2[:], xT_all[:, j, i * T:(i + 1) * T],
                                         wv_sb[:, j, 512:768],
                                         start=(j == 0), stop=(j == ND - 1))
                    nc.vector.tensor_copy(v_sb[:, i, 0:512], pv5[:])
                    nc.scalar.copy(v_sb[:, i, 512:768], pv2[:])

            # ====== attention + B1 (out-proj + LN1 + x1^T), interleaved =====
            # PSUM budget (8 banks): st 2x2 + yp 1 + tr 1 + pz 2 = 8
            # QK matmuls grouped by operand partition offset: a po=0 -> 64
            # transition between consecutive matmuls into the same PSUM bank
            # faults on HW, so even heads (po=0) fill bank 0, odd heads bank 1.
            ORDER = (0, 2, 1, 3)
            RPOS = {0: 0, 2: 1, 1: 2, 3: 3}

            def emit_attn(t, attn, psS):
                m01 = m01f_sb if t == 0 else m01r_sb
                y_blk = attn.tile([T, D], bf16, tag="yblk")
                for c3 in range(3):                      # 4 heads per chunk
                    ps_st = psS.tile([T, 4, 2, T], f32, tag="st", bufs=2)
                    for ri, h4 in enumerate(ORDER):
                        h = c3 * 4 + h4
                        cc, po = h // 2, (h % 2) * HD
                        for kb in range(2):
                            nc.tensor.matmul(
                                ps_st[:, ri, kb, :],
                                kT[po:po + HD, cc, (t + kb) * T:(t + kb + 1) * T],
                                qT[po:po + HD, cc, t * T:(t + 1) * T],
                                start=True, stop=True)
                    P = attn.tile([T, 4, 2, T], bf16, tag="P")
                    nc.scalar.activation(P[:], ps_st[:], AF.Exp, scale=0.125)
                    nc.vector.tensor_tensor(P[:], P[:], m01[:], op=ALU.mult)
                    yp = psS.tile([T, 4, 80], f32, tag="yp", bufs=1)
                    nmm = 0
                    for h4 in range(4):
                        h = c3 * 4 + h4
                        ri = RPOS[h4]
                        for kb in range(2):
                            nc.tensor.matmul(
                                yp[:, h4, 0:HD], P[:, ri, kb, :],
                                v_sb[:, t + kb, h * HD:(h + 1) * HD],
                                start=(nmm == 0), stop=False,
                                skip_group_check=True)
                            nmm += 1
                            nc.tensor.matmul(
                                yp[:, h4, HD:HD + 1], P[:, ri, kb, :],
                                ones_col[:],
                                start=False, stop=(nmm == 7),
                                skip_group_check=True)
                            nmm += 1
                    rec = attn.tile([T, 4], f32, tag="rec")
                    nc.vector.reciprocal(rec[:], yp[:, :, HD])
                    for h4 in range(4):
                        h = c3 * 4 + h4
                        nc.vector.tensor_scalar_mul(
                            y_blk[:, h * HD:(h + 1) * HD], yp[:, h4, 0:HD],
                            rec[:, h4:h4 + 1])
                ptr = psS.tile([T, ND, T], bf16, tag="tr", bufs=1)
                for j in range(ND):
                    nc.tensor.transpose(ptr[:, j, :],
                                        y_blk[:, j * T:(j + 1) * T], ident[:])
                nc.vector.tensor_copy(yT_all[:, t, :, :], ptr[:])

            def emit_b1_mm(t, wb, psB):
                pz = psB.tile([T, D], f32, tag="pz", bufs=1)
                for j in range(ND):
                    nc.tensor.matmul(pz[:, 0:512], yT_all[:, t, j, :],
                                     wo_sb[:, j, 0:512],
                                     start=(j == 0), stop=False)
                nc.tensor.matmul(pz[:, 0:512], e0mat[:], obrep_sb[:, 0:512],
                                 start=False, stop=True)
                for j in range(ND):
                    nc.tensor.matmul(pz[:, 512:768], yT_all[:, t, j, :],
                                     wo_sb[:, j, 512:768],
                                     start=(j == 0), stop=False)
                nc.tensor.matmul(pz[:, 512:768], e0mat[:],
                                 obrep_sb[:, 512:768],
                                 start=False, stop=True)
                return pz

            def emit_b1_res(t, wb, pz):
                x1pre = wb.tile([T, D], f32, tag="x1pre")
                s1a = wb.tile([T, 1], f32, tag="s1a")
                s1b = wb.tile([T, 1], f32, tag="s1b")
                nc.vector.scalar_tensor_tensor(
                    x1pre[:, 0:512], pz[:, 0:512], 1.0, xh_sb[:, t + 1, 0:512],
                    op0=ALU.mult, op1=ALU.add, accum_out=s1a[:])
                nc.vector.scalar_tensor_tensor(
                    x1pre[:, 512:768], pz[:, 512:768], 1.0,
                    xh_sb[:, t + 1, 512:768],
                    op0=ALU.mult, op1=ALU.add, accum_out=s1b[:])
                return x1pre, s1a, s1b

            def emit_b1_ln(t, wb, psB, x1pre, s1a, s1b):
                s1 = wb.tile([T, 1], f32, tag="s1")
                nc.vector.tensor_tensor(s1[:], s1a[:], s1b[:], op=ALU.add)
                nm = wb.tile([T, 1], f32, tag="nm")
                nc.scalar.mul(nm[:], s1[:], -1.0 / D)
                xc = wb.tile([T, D], f32, tag="xc")
                nc.vector.tensor_scalar_add(xc[:], x1pre[:], nm[:])
                sq = wb.tile([T, D], f32, tag="sq")
                vs = wb.tile([T, 1], f32, tag="vs")
                nc.vector.scalar_tensor_tensor(sq[:], xc[:], 1.0, xc[:],
                                               op0=ALU.mult, op1=ALU.mult,
                                               accum_out=vs[:])
                std = wb.tile([T, 1], f32, tag="std")
                nc.scalar.activation(std[:], vs[:], AF.Sqrt, bias=eps_sb[:],
                                     scale=1.0 / D)
                rstd = wb.tile([T, 1], f32, tag="rstd")
                nc.vector.reciprocal(rstd[:], std[:])
                nc.vector.scalar_tensor_tensor(x1_all[:, t, :], xc[:], rstd[:],
                                               g1rep_sb[:], op0=ALU.mult,
                                               op1=ALU.mult)
                ptr = psB.tile([T, ND, T], bf16, tag="tr", bufs=1)
                for j in range(ND):
                    nc.tensor.transpose(ptr[:, j, :],
                                        x1_all[:, t, j * T:(j + 1) * T],
                                        ident[:])
                nc.scalar.copy(x1T_all[:, :, t * T:(t + 1) * T], ptr[:])

            with tc.tile_pool(name="attn", bufs=3) as attn, \
                 tc.tile_pool(name="wb1w", bufs=2) as wb, \
                 tc.tile_pool(name="psS", bufs=1, space="PSUM") as psS:
                def emit_b1(t):
                    pz = emit_b1_mm(t, wb, psS)
                    x1pre, s1a, s1b = emit_b1_res(t, wb, pz)
                    emit_b1_ln(t, wb, psS, x1pre, s1a, s1b)

                emit_attn(0, attn, psS)
                for t in range(1, NB):
                    emit_attn(t, attn, psS)
                    emit_b1(t - 1)
                emit_b1(NB - 1)

        # ============= phase B2/B3: FFN + LN2, stage-major ==============
        with tc.tile_pool(name="wB2", bufs=1) as wB2, \
             tc.tile_pool(name="hbuf", bufs=1) as hbuf, \
             tc.tile_pool(name="workB", bufs=2) as workB, \
             tc.tile_pool(name="psF", bufs=2, space="PSUM") as psF:
            w2_sb = wB2.tile([T, NF, D], bf16, tag="w2")
            w2r = w2_d.rearrange("(j p) n -> p j n", p=T)
            nc.sync.dma_start(w2_sb[:, 0:12, :], w2r[:, 0:12, :])
            nc.sync.dma_start(w2_sb[:, 12:24, :], w2r[:, 12:24, :])

            def emit_b2(g, h_g):
                for fi in range(NF):
                    ph = psF.tile([T, 512], f32, tag="ph")
                    for j in range(ND):
                        nc.tensor.matmul(
                            ph[:], w1_sb[:, j, fi * T:(fi + 1) * T],
                            x1T_all[:, j, g * 512:(g + 1) * 512],
                            start=(j == 0), stop=(j == ND - 1))
                    act_fn = AF.Gelu if act == "gelu" else AF.Identity
                    nc.scalar.activation(h_g[:, fi, :], ph[:], act_fn,
                                         bias=gelub_sb[:, fi:fi + 1])

            def emit_b3(t, h_g):
                px5 = psF.tile([T, 512], f32, tag="px5")
                px2 = psF.tile([T, 256], f32, tag="px2")
                tin = (t % 4) * T
                for fi in range(NF):
                    nc.tensor.matmul(px5[:], h_g[:, fi, tin:tin + T],
                                     w2_sb[:, fi, 0:512],
                                     start=(fi == 0), stop=False)
                nc.tensor.matmul(px5[:], e0mat[:], b2rep_sb[:, 0:512],
                                 start=False, stop=True)
                for fi in range(NF):
                    nc.tensor.matmul(px2[:], h_g[:, fi, tin:tin + T],
                                     w2_sb[:, fi, 512:768],
                                     start=(fi == 0), stop=False)
                nc.tensor.matmul(px2[:], e0mat[:], b2rep_sb[:, 512:768],
                                 start=False, stop=True)
                x2pre = workB.tile([T, D], f32, tag="x2pre")
                s1a = workB.tile([T, 1], f32, tag="s1a")
                s1b = workB.tile([T, 1], f32, tag="s1b")
                nc.vector.scalar_tensor_tensor(
                    x2pre[:, 0:512], px5[:], 1.0, x1_all[:, t, 0:512],
                    op0=ALU.mult, op1=ALU.add, accum_out=s1a[:])
                nc.vector.scalar_tensor_tensor(
                    x2pre[:, 512:768], px2[:], 1.0, x1_all[:, t, 512:768],
                    op0=ALU.mult, op1=ALU.add, accum_out=s1b[:])
                s1 = workB.tile([T, 1], f32, tag="s1")
                nc.vector.tensor_tensor(s1[:], s1a[:], s1b[:], op=ALU.add)
                nm = workB.tile([T, 1], f32, tag="nm")
                nc.scalar.mul(nm[:], s1[:], -1.0 / D)
                xc = workB.tile([T, D], f32, tag="xc")
                nc.vector.tensor_scalar_add(xc[:], x2pre[:], nm[:])
                sq = workB.tile([T, D], f32, tag="sq")
                vs = workB.tile([T, 1], f32, tag="vs")
                nc.vector.scalar_tensor_tensor(sq[:], xc[:], 1.0, xc[:],
                                               op0=ALU.mult, op1=ALU.mult,
                                               accum_out=vs[:])
                std = workB.tile([T, 1], f32, tag="std")
                nc.scalar.activation(std[:], vs[:], AF.Sqrt, bias=eps_sb[:],
                                     scale=1.0 / D)
                rstd = workB.tile([T, 1], f32, tag="rstd")
                nc.vector.reciprocal(rstd[:], std[:])
                xg = workB.tile([T, D], f32, tag="xg")
                nc.vector.scalar_tensor_tensor(xg[:], xc[:], rstd[:],
                                               g2rep_sb[:], op0=ALU.mult,
                                               op1=ALU.mult)
                ob = workB.tile([T, D], f32, tag="outb")
                nc.vector.tensor_tensor(ob[:], xg[:], n2brep_sb[:],
                                        op=ALU.add)
                nc.sync.dma_start(out_d[t * T:(t + 1) * T, :], ob[:])

            h_g0 = hbuf.tile([T, NF, 512], bf16, tag="h")
            emit_b2(0, h_g0)
            for t in range(4):
                emit_b3(t, h_g0)
            h_g1 = hbuf.tile([T, NF, 512], bf16, tag="h")
            emit_b2(1, h_g1)
            for t in range(4, NB):
                emit_b3(t, h_g1)

    nc.compile()
    return nc


def _get_nc(act="gelu"):
    if act not in _CACHE:
        _CACHE[act] = _build_nc(act)
    return _CACHE[act]


def make_in_maps(x, in_proj_w, in_proj_b, out_w, out_b, ff_w1, ff_b1,
                 ff_w2, ff_b2, n1_g, n1_b, n2_g, n2_b):
    bf = ml_dtypes.bfloat16
    f32 = np.float32
    x = np.asarray(x, f32).reshape(B, S, D)
    in_proj_w = np.asarray(in_proj_w, f32)
    in_proj_b = np.asarray(in_proj_b, f32)
    out_w = np.asarray(out_w, f32)
    ff_w1 = np.asarray(ff_w1, f32)
    ff_w2 = np.asarray(ff_w2, f32)
    n1_b = np.asarray(n1_b, f32)

    v_b = in_proj_b[DQK:]
    ob_eff = np.asarray(out_b, f32) + v_b @ out_w.T          # V-bias folded
    gelub_eff = np.asarray(ff_b1, f32) + n1_b @ ff_w1.T      # n1_b folded
    b2row_eff = np.asarray(ff_b2, f32) + n1_b                # n1_b residual

    shared = {
        "wqkT": np.ascontiguousarray(in_proj_w[:DQK].T).astype(bf),
        "wvT": np.ascontiguousarray(in_proj_w[DQK:].T).astype(bf),
        "woT": np.ascontiguousarray(out_w.T).astype(bf),
        "w1T": np.ascontiguousarray(ff_w1.T).astype(bf),
        "w2T": np.ascontiguousarray(ff_w2.T).astype(bf),
        "qkb": np.ascontiguousarray(
            in_proj_b[:DQK].reshape(2 * ND, T).T),
        "gelub": np.ascontiguousarray(gelub_eff.reshape(NF, T).T),
        "obrep": np.ascontiguousarray(
            np.broadcast_to(ob_eff[None, :], (T, D))).astype(bf),
        "b2rep": np.ascontiguousarray(
            np.broadcast_to(b2row_eff[None, :], (T, D))).astype(bf),
        "g1rep": np.ascontiguousarray(
            np.broadcast_to(np.asarray(n1_g, f32)[None, :], (T, D))),
        "g2rep": np.ascontiguousarray(
            np.broadcast_to(np.asarray(n2_g, f32)[None, :], (T, D))),
        "n2brep": np.ascontiguousarray(
            np.broadcast_to(np.asarray(n2_b, f32)[None, :], (T, D))),
    }

    # masks in [k, (h4, kb, q)] layout, 0/1 bf16, tiled over 4 heads
    k_i = np.arange(T, dtype=np.int64)[:, None]
    q_i = np.arange(T, dtype=np.int64)[None, :]
    m_kb0 = (k_i > q_i).astype(f32)         # previous key block
    m_kb1 = (k_i <= q_i).astype(f32)        # current key block (causal)
    rest = np.concatenate([m_kb0, m_kb1], axis=1)          # [T, 2T]
    first = np.concatenate([np.zeros((T, T), f32), m_kb1], axis=1)
    m01r = np.ascontiguousarray(np.tile(rest, (1, 4))).astype(bf)
    m01f_bs = np.ascontiguousarray(np.tile(first, (1, 4))).astype(bf)

    in_maps = []
    for c in range(N_CORES):
        b, i0 = divmod(c * NTOK, S)
        halo = (np.zeros((T, D), f32) if i0 == 0 else x[b, i0 - T:i0])
        xh = np.ascontiguousarray(
            np.concatenate([halo, x[b, i0:i0 + NTOK]], axis=0)).astype(bf)
        m = dict(shared)
        m["xh"] = xh
        m["m01f"] = m01f_bs if i0 == 0 else m01r
        m["m01r"] = m01r
        in_maps.append(m)
    return in_maps


def kernel(**inputs):
    from concourse.bass_utils import run_bass_kernel_spmd
    nc = _get_nc()
    in_maps = make_in_maps(**inputs)
    res = run_bass_kernel_spmd(nc, in_maps, core_ids=list(range(N_CORES)))
    outs = [res.results[c]["out"] for c in range(N_CORES)]
    return np.concatenate(outs, axis=0).reshape(B, S, D).astype(np.float32)


# revision 22
# speedup vs baseline: 1.0941x; 1.0301x over previous
"""Trainium2 Bass kernel for a local-attention transformer block.

Data-parallel over tokens: 8 shards of 1024 tokens (+128-token halo).
Per core: transpose x to [d,tok]; QKV with large moving dims; attention
in transposed-score orientation (exp emits P^T directly, denominator via
an appended ones-column in the PV matmul, mask applied multiplicatively
after exp); out-proj + LN1 interleaved with attention; FFN with N=512
moving dims and gelu bias folded with n1_b@W1^T. Biases enter PSUM as
rank-1 matmuls (full-K E0 selector; K=1 lhsT and partition-offset
transitions between consecutive matmuls into one PSUM bank both fault
on HW, so QK matmuls are also grouped by operand partition offset).
Matmuls bf16/fp32-accum; softmax fp32; layernorm centering in bf16 with
fp32 statistics.
"""

import numpy as np
import ml_dtypes

# ---- problem constants (hardcoded per contract) ----
B, S, D = 2, 4096, 768
NH, HD = 12, 64
DFF = 4 * D            # 3072
DQK = 2 * D            # 1536
WIN = 128
EPS = 1e-5
T = 128
NB = 8                 # own 128-token blocks per core
NBH = NB + 1           # with halo block
NTOK = NB * T          # 1024
NTOKH = NBH * T        # 1152
ND = D // T            # 6
NF = DFF // T          # 24
N_CORES = 8

_CACHE = {}


def _build_nc(act="gelu"):
    import concourse.bacc as bacc
    import concourse.mybir as mybir
    from concourse import tile
    from concourse.masks import make_identity

    f32 = mybir.dt.float32
    bf16 = mybir.dt.bfloat16
    AF = mybir.ActivationFunctionType
    ALU = mybir.AluOpType

    nc = bacc.Bacc("TRN2", target_bir_lowering=False, debug=False,
                   num_devices=N_CORES)

    # ---- DRAM I/O ----
    xh_d = nc.dram_tensor("xh", [NTOKH, D], bf16, kind="ExternalInput").ap()
    wqk_d = nc.dram_tensor("wqkT", [D, DQK], bf16, kind="ExternalInput").ap()
    wv_d = nc.dram_tensor("wvT", [D, D], bf16, kind="ExternalInput").ap()
    wo_d = nc.dram_tensor("woT", [D, D], bf16, kind="ExternalInput").ap()
    w1_d = nc.dram_tensor("w1T", [D, DFF], bf16, kind="ExternalInput").ap()
    w2_d = nc.dram_tensor("w2T", [DFF, D], bf16, kind="ExternalInput").ap()
    qkb_d = nc.dram_tensor("qkb", [T, 2 * ND], f32, kind="ExternalInput").ap()
    gelub_d = nc.dram_tensor("gelub", [T, NF], f32, kind="ExternalInput").ap()
    obrep_d = nc.dram_tensor("obrep", [T, D], bf16, kind="ExternalInput").ap()
    b2rep_d = nc.dram_tensor("b2rep", [T, D], bf16, kind="ExternalInput").ap()
    g1rep_d = nc.dram_tensor("g1rep", [T, D], bf16, kind="ExternalInput").ap()
    g2rep_d = nc.dram_tensor("g2rep", [T, D], bf16, kind="ExternalInput").ap()
    n2brep_d = nc.dram_tensor("n2brep", [T, D], f32, kind="ExternalInput").ap()
    # masks in transposed [k, (h4, kb, q)] orientation, 0/1, tiled for 4 heads
    m01f_d = nc.dram_tensor("m01f", [T, 4 * 2 * T], bf16, kind="ExternalInput").ap()
    m01r_d = nc.dram_tensor("m01r", [T, 4 * 2 * T], bf16, kind="ExternalInput").ap()
    out_d = nc.dram_tensor("out", [NTOK, D], f32, kind="ExternalOutput").ap()

    with tile.TileContext(nc) as tc:
      with tc.tile_pool(name="persist", bufs=1) as persist, \
           tc.tile_pool(name="mid", bufs=1) as mid:
        ident = persist.tile([T, T], bf16, tag="ident")
        make_identity(nc, ident[:])
        ones_col = persist.tile([T, 1], bf16, tag="ones_col")
        nc.gpsimd.memset(ones_col[:], 1.0)
        # E0: row 0 all-ones selector for rank-1 bias adds (K=128)
        e0mat = persist.tile([T, T], bf16, tag="e0mat")
        nc.gpsimd.memset(e0mat[:], 0.0)
        nc.gpsimd.memset(e0mat[0:1, :], 1.0)
        eps_sb = persist.tile([T, 1], f32, tag="eps")
        nc.gpsimd.memset(eps_sb[:], EPS)
        qkb_sb = persist.tile([T, 2 * ND], f32, tag="qkb")
        gelub_sb = persist.tile([T, NF], f32, tag="gelub")
        obrep_sb = persist.tile([T, D], bf16, tag="obrep")
        b2rep_sb = persist.tile([T, D], bf16, tag="b2rep")
        g1rep_sb = persist.tile([T, D], bf16, tag="g1rep")
        g2rep_sb = persist.tile([T, D], bf16, tag="g2rep")
        n2brep_sb = persist.tile([T, D], f32, tag="n2brep")
        m01f_sb = persist.tile([T, 8 * T], bf16, tag="m01f")
        m01r_sb = persist.tile([T, 8 * T], bf16, tag="m01r")
        xh_sb = persist.tile([T, NBH, D], bf16, tag="xh")
        # xh leads the DMA queue (phase A blocks on it), then the small
        # Q/K bias; remaining consts follow the phase-A weights.
        nc.sync.dma_start(xh_sb[:], xh_d.rearrange("(b p) d -> p b d", p=T))
        nc.sync.dma_start(qkb_sb[:], qkb_d[:])

        x1_all = mid.tile([T, NB, D], bf16, tag="x1")
        x1T_all = mid.tile([T, ND, NTOK], bf16, tag="x1T")

        with tc.tile_pool(name="attnp", bufs=2) as attn, \
             tc.tile_pool(name="wb", bufs=2) as wb, \
             tc.tile_pool(name="psS", bufs=1, space="PSUM") as psS, \
             tc.tile_pool(name="pwoyT", bufs=1) as pwoyT:
            wo_sb = pwoyT.tile([T, ND, D], bf16, tag="wo")
            yT_all = pwoyT.tile([T, NB, ND, T], bf16, tag="yT")
            w1_sb = pwoyT.tile([T, ND, DFF], bf16, tag="w1")

            with tc.tile_pool(name="pqkv", bufs=1) as pqkv:
                qT = pqkv.tile([T, ND, NTOK], bf16, tag="qT")
                kT = pqkv.tile([T, ND, NTOKH], bf16, tag="kT")
                v_sb = pqkv.tile([T, NBH, D], bf16, tag="v")

                # ============== phase A: x^T, then Q/K/V ==================
                with tc.tile_pool(name="pa", bufs=1) as pa:
                    wqk_sb = pa.tile([T, ND, DQK], bf16, tag="wqk")
                    nc.sync.dma_start(wqk_sb[:],
                                      wqk_d.rearrange("(j p) n -> p j n", p=T))
                    wv_sb = pa.tile([T, ND, D], bf16, tag="wv")
                    nc.sync.dma_start(wv_sb[:],
                                      wv_d.rearrange("(j p) n -> p j n", p=T))
                    # later-phase weights + consts queue behind phase-A ones
                    nc.sync.dma_start(wo_sb[:],
                                      wo_d.rearrange("(j p) n -> p j n", p=T))
                    nc.sync.dma_start(w1_sb[:],
                                      w1_d.rearrange("(j p) n -> p j n", p=T))
                    nc.sync.dma_start(m01f_sb[:], m01f_d[:])
                    nc.sync.dma_start(m01r_sb[:], m01r_d[:])
                    nc.sync.dma_start(g1rep_sb[:], g1rep_d[:])
                    nc.sync.dma_start(obrep_sb[:], obrep_d[:])
                    nc.sync.dma_start(gelub_sb[:], gelub_d[:])
                    nc.sync.dma_start(b2rep_sb[:], b2rep_d[:])
                    nc.sync.dma_start(g2rep_sb[:], g2rep_d[:])
                    nc.sync.dma_start(n2brep_sb[:], n2brep_d[:])
                    xT_all = pa.tile([T, ND, NTOKH], bf16, tag="xT")

                    for i in range(NBH):
                        ptr = psS.tile([T, ND, T], bf16, tag="tr", bufs=1)
                        for j in range(ND):
                            nc.tensor.transpose(ptr[:, j, :],
                                                xh_sb[:, i, j * T:(j + 1) * T],
                                                ident[:])
                        nc.scalar.copy(xT_all[:, :, i * T:(i + 1) * T], ptr[:])

                    # Q: own tokens only (2 groups of 512)
                    for g in range(2):
                        tsl = slice(T + g * 512, T + (g + 1) * 512)
                        osl = slice(g * 512, (g + 1) * 512)
                        for cc in range(ND):
                            pq = psS.tile([T, 512], f32, tag="st", bufs=2)
                            for j in range(ND):
                                nc.tensor.matmul(
                                    pq[:], wqk_sb[:, j, cc * T:(cc + 1) * T],
                                    xT_all[:, j, tsl],
                                    start=(j == 0), stop=(j == ND - 1))
                            nc.vector.tensor_scalar_add(qT[:, cc, osl], pq[:],
                                                        qkb_sb[:, cc:cc + 1])
                    # K: halo'd tokens (3 groups of 384)
                    for g in range(3):
                        tsl = slice(g * 384, (g + 1) * 384)
                        for cc in range(ND):
                            pk = psS.tile([T, 384], f32, tag="st", bufs=2)
                            for j in range(ND):
                                nc.tensor.matmul(
                                    pk[:],
                                    wqk_sb[:, j, D + cc * T:D + (cc + 1) * T],
                                    xT_all[:, j, tsl],
                                    start=(j == 0), stop=(j == ND - 1))
                            nc.scalar.activation(
                                kT[:, cc, tsl], pk[:], AF.Identity,
                                bias=qkb_sb[:, ND + cc:ND + cc + 1])
                    # V: [tok, ch] layout per block (bias folded into obrep)
                    for i in range(NBH):
                        pv5 = psS.tile([T, 512], f32, tag="st", bufs=2)
                        pv2 = psS.tile([T, D], f32, tag="pz", bufs=1)
                        for j in range(ND):
                            nc.tensor.matmul(pv5[:],
                                             xT_all[:, j, i * T:(i + 1) * T],
                                             wv_sb[:, j, 0:512],
                                             start=(j == 0), stop=(j == ND - 1))
                        for j in range(ND):
                            nc.tensor.matmul(pv2[:, 0:256],
                                             xT_all[:, j, i * T:(i + 1) * T],
                                             wv_sb[:, j, 512:768],
                                             start=(j == 0), stop=(j == ND - 1))
                        nc.vector.tensor_copy(v_sb[:, i, 0:512], pv5[:])
                        nc.scalar.copy(v_sb[:, i, 512:768], pv2[:, 0:256])

                # === attention + B1 (out-proj + LN1 + x1^T), interleaved ===
                # PSUM budget (8 banks): st 2x2 + yp 1 + tr 1 + pz 2 = 8
                # QK matmuls grouped by operand partition offset: a po=0 -> 64
                # transition between consecutive matmuls into the same PSUM
                # bank faults on HW; even heads fill bank 0, odd heads bank 1.
                ORDER = (0, 2, 1, 3)
                RPOS = {0: 0, 2: 1, 1: 2, 3: 3}

                def emit_attn(t):
                    m01 = m01f_sb if t == 0 else m01r_sb
                    y_blk = attn.tile([T, D], bf16, tag="yblk")
                    for c3 in range(3):                  # 4 heads per chunk
                        ps_st = psS.tile([T, 4, 2, T], f32, tag="st", bufs=2)
                        for ri, h4 in enumerate(ORDER):
                            h = c3 * 4 + h4
                            cc, po = h // 2, (h % 2) * HD
                            for kb in range(2):
                                nc.tensor.matmul(
                                    ps_st[:, ri, kb, :],
                                    kT[po:po + HD, cc,
                                       (t + kb) * T:(t + kb + 1) * T],
                                    qT[po:po + HD, cc, t * T:(t + 1) * T],
                                    start=True, stop=True)
                        P = attn.tile([T, 4, 2, T], bf16, tag="P", bufs=2)
                        nc.scalar.activation(P[:], ps_st[:], AF.Exp, scale=0.125)
                        nc.vector.tensor_tensor(P[:], P[:], m01[:], op=ALU.mult)
                        yp = psS.tile([T, 4, 80], f32, tag="yp", bufs=1)
                        nmm = 0
                        for h4 in range(4):
                            h = c3 * 4 + h4
                            ri = RPOS[h4]
                            for kb in range(2):
                                nc.tensor.matmul(
                                    yp[:, h4, 0:HD], P[:, ri, kb, :],
                                    v_sb[:, t + kb, h * HD:(h + 1) * HD],
                                    start=(nmm == 0), stop=False,
                                    skip_group_check=True)
                                nmm += 1
                                nc.tensor.matmul(
                                    yp[:, h4, HD:HD + 1], P[:, ri, kb, :],
                                    ones_col[:],
                                    start=False, stop=(nmm == 7),
                                    skip_group_check=True)
                                nmm += 1
                        rec = attn.tile([T, 4], f32, tag="rec")
                        nc.vector.reciprocal(rec[:], yp[:, :, HD])
                        for h4 in range(4):
                            h = c3 * 4 + h4
                            nc.vector.tensor_scalar_mul(
                                y_blk[:, h * HD:(h + 1) * HD], yp[:, h4, 0:HD],
                                rec[:, h4:h4 + 1])
                    ptr = psS.tile([T, ND, T], bf16, tag="tr", bufs=1)
                    for j in range(ND):
                        nc.tensor.transpose(ptr[:, j, :],
                                            y_blk[:, j * T:(j + 1) * T],
                                            ident[:])
                    nc.vector.tensor_copy(yT_all[:, t, :, :], ptr[:])

                def emit_b1(t):
                    pz = psS.tile([T, D], f32, tag="pz", bufs=1)
                    for j in range(ND):
                        nc.tensor.matmul(pz[:, 0:512], yT_all[:, t, j, :],
                                         wo_sb[:, j, 0:512],
                                         start=(j == 0), stop=False)
                    nc.tensor.matmul(pz[:, 0:512], e0mat[:], obrep_sb[:, 0:512],
                                     start=False, stop=True)
                    for j in range(ND):
                        nc.tensor.matmul(pz[:, 512:768], yT_all[:, t, j, :],
                                         wo_sb[:, j, 512:768],
                                         start=(j == 0), stop=False)
                    nc.tensor.matmul(pz[:, 512:768], e0mat[:],
                                     obrep_sb[:, 512:768],
                                     start=False, stop=True)
                    x1pre = wb.tile([T, D], bf16, tag="x1pre")
                    s1a = wb.tile([T, 1], f32, tag="s1a")
                    s1b = wb.tile([T, 1], f32, tag="s1b")
                    nc.vector.scalar_tensor_tensor(
                        x1pre[:, 0:512], pz[:, 0:512], 1.0,
                        xh_sb[:, t + 1, 0:512],
                        op0=ALU.mult, op1=ALU.add, accum_out=s1a[:])
                    nc.vector.scalar_tensor_tensor(
                        x1pre[:, 512:768], pz[:, 512:768], 1.0,
                        xh_sb[:, t + 1, 512:768],
                        op0=ALU.mult, op1=ALU.add, accum_out=s1b[:])
                    # LN1 (mean from accumulated sums; bf16 data passes)
                    s1 = wb.tile([T, 1], f32, tag="s1")
                    nc.vector.tensor_tensor(s1[:], s1a[:], s1b[:], op=ALU.add)
                    nm = wb.tile([T, 1], f32, tag="nm")
                    nc.scalar.mul(nm[:], s1[:], -1.0 / D)
                    xc = wb.tile([T, D], bf16, tag="xc")
                    nc.vector.tensor_scalar_add(xc[:], x1pre[:], nm[:])
                    vs = wb.tile([T, 1], f32, tag="vs")
                    nc.vector.scalar_tensor_tensor(x1pre[:], xc[:], 1.0, xc[:],
                                                   op0=ALU.mult, op1=ALU.mult,
                                                   accum_out=vs[:])
                    std = wb.tile([T, 1], f32, tag="std")
                    nc.scalar.activation(std[:], vs[:], AF.Sqrt, bias=eps_sb[:],
                                         scale=1.0 / D)
                    rstd = wb.tile([T, 1], f32, tag="rstd")
                    nc.vector.reciprocal(rstd[:], std[:])
                    nc.vector.scalar_tensor_tensor(x1_all[:, t, :], xc[:],
                                                   rstd[:], g1rep_sb[:],
                                                   op0=ALU.mult, op1=ALU.mult)
                    ptr = psS.tile([T, ND, T], bf16, tag="tr", bufs=1)
                    for j in range(ND):
                        nc.tensor.transpose(ptr[:, j, :],
                                            x1_all[:, t, j * T:(j + 1) * T],
                                            ident[:])
                    nc.scalar.copy(x1T_all[:, :, t * T:(t + 1) * T], ptr[:])

                emit_attn(0)
                for t in range(1, NB):
                    emit_attn(t)
                    emit_b1(t - 1)
                emit_b1(NB - 1)

            # ========== phase B2/B3: FFN + LN2 (pools stay open) ==========
            with tc.tile_pool(name="wB2", bufs=1) as wB2, \
                 tc.tile_pool(name="hbuf", bufs=1) as hbuf:
                w2_sb = wB2.tile([T, NF, D], bf16, tag="w2")
                w2r = w2_d.rearrange("(j p) n -> p j n", p=T)
                nc.sync.dma_start(w2_sb[:, 0:12, :], w2r[:, 0:12, :])
                nc.sync.dma_start(w2_sb[:, 12:24, :], w2r[:, 12:24, :])

                def emit_b2(g, h_g):
                    for fi in range(NF):
                        ph = psS.tile([T, 512], f32, tag="st", bufs=2)
                        for j in range(ND):
                            nc.tensor.matmul(
                                ph[:], w1_sb[:, j, fi * T:(fi + 1) * T],
                                x1T_all[:, j, g * 512:(g + 1) * 512],
                                start=(j == 0), stop=(j == ND - 1))
                        act_fn = AF.Gelu if act == "gelu" else AF.Identity
                        nc.scalar.activation(h_g[:, fi, :], ph[:], act_fn,
                                             bias=gelub_sb[:, fi:fi + 1])

                def emit_b3(t, h_g):
                    px = psS.tile([T, D], f32, tag="pz", bufs=1)
                    tin = (t % 4) * T
                    for fi in range(NF):
                        nc.tensor.matmul(px[:, 0:512], h_g[:, fi, tin:tin + T],
                                         w2_sb[:, fi, 0:512],
                                         start=(fi == 0), stop=False)
                    nc.tensor.matmul(px[:, 0:512], e0mat[:], b2rep_sb[:, 0:512],
                                     start=False, stop=True)
                    for fi in range(NF):
                        nc.tensor.matmul(px[:, 512:768],
                                         h_g[:, fi, tin:tin + T],
                                         w2_sb[:, fi, 512:768],
                                         start=(fi == 0), stop=False)
                    nc.tensor.matmul(px[:, 512:768], e0mat[:],
                                     b2rep_sb[:, 512:768],
                                     start=False, stop=True)
                    x2pre = wb.tile([T, D], bf16, tag="x1pre")
                    s1a = wb.tile([T, 1], f32, tag="s1a")
                    s1b = wb.tile([T, 1], f32, tag="s1b")
                    nc.vector.scalar_tensor_tensor(
                        x2pre[:, 0:512], px[:, 0:512], 1.0,
                        x1_all[:, t, 0:512],
                        op0=ALU.mult, op1=ALU.add, accum_out=s1a[:])
                    nc.vector.scalar_tensor_tensor(
                        x2pre[:, 512:768], px[:, 512:768], 1.0,
                        x1_all[:, t, 512:768],
                        op0=ALU.mult, op1=ALU.add, accum_out=s1b[:])
                    s1 = wb.tile([T, 1], f32, tag="s1")
                    nc.vector.tensor_tensor(s1[:], s1a[:], s1b[:], op=ALU.add)
                    nm = wb.tile([T, 1], f32, tag="nm")
                    nc.scalar.mul(nm[:], s1[:], -1.0 / D)
                    xc = wb.tile([T, D], bf16, tag="xc")
                    nc.vector.tensor_scalar_add(xc[:], x2pre[:], nm[:])
                    vs = wb.tile([T, 1], f32, tag="vs")
                    nc.vector.scalar_tensor_tensor(x2pre[:], xc[:], 1.0, xc[:],
                                                   op0=ALU.mult, op1=ALU.mult,
                                                   accum_out=vs[:])
                    std = wb.tile([T, 1], f32, tag="std")
                    nc.scalar.activation(std[:], vs[:], AF.Sqrt, bias=eps_sb[:],
                                         scale=1.0 / D)
                    rstd = wb.tile([T, 1], f32, tag="rstd")
                    nc.vector.reciprocal(rstd[:], std[:])
                    ob = wb.tile([T, D], f32, tag="outb")
                    nc.vector.scalar_tensor_tensor(ob[:], xc[:], rstd[:],
                                                   g2rep_sb[:], op0=ALU.mult,
                                                   op1=ALU.mult)
                    nc.vector.tensor_tensor(ob[:], ob[:], n2brep_sb[:],
                                            op=ALU.add)
                    nc.sync.dma_start(out_d[t * T:(t + 1) * T, :], ob[:])

                h_g0 = hbuf.tile([T, NF, 512], bf16, tag="h")
                emit_b2(0, h_g0)
                for t in range(4):
                    emit_b3(t, h_g0)
                h_g1 = hbuf.tile([T, NF, 512], bf16, tag="h")
                emit_b2(1, h_g1)
                for t in range(4, NB):
                    emit_b3(t, h_g1)

    nc.compile()
    return nc


def _get_nc(act="gelu"):
    if act not in _CACHE:
        _CACHE[act] = _build_nc(act)
    return _CACHE[act]


def make_in_maps(x, in_proj_w, in_proj_b, out_w, out_b, ff_w1, ff_b1,
                 ff_w2, ff_b2, n1_g, n1_b, n2_g, n2_b):
    bf = ml_dtypes.bfloat16
    f32 = np.float32
    x = np.asarray(x, f32).reshape(B, S, D)
    in_proj_w = np.asarray(in_proj_w, f32)
    in_proj_b = np.asarray(in_proj_b, f32)
    out_w = np.asarray(out_w, f32)
    ff_w1 = np.asarray(ff_w1, f32)
    ff_w2 = np.asarray(ff_w2, f32)
    n1_b = np.asarray(n1_b, f32)

    v_b = in_proj_b[DQK:]
    ob_eff = np.asarray(out_b, f32) + v_b @ out_w.T          # V-bias folded
    gelub_eff = np.asarray(ff_b1, f32) + n1_b @ ff_w1.T      # n1_b folded
    b2row_eff = np.asarray(ff_b2, f32) + n1_b                # n1_b residual

    shared = {
        "wqkT": np.ascontiguousarray(in_proj_w[:DQK].T).astype(bf),
        "wvT": np.ascontiguousarray(in_proj_w[DQK:].T).astype(bf),
        "woT": np.ascontiguousarray(out_w.T).astype(bf),
        "w1T": np.ascontiguousarray(ff_w1.T).astype(bf),
        "w2T": np.ascontiguousarray(ff_w2.T).astype(bf),
        "qkb": np.ascontiguousarray(
            in_proj_b[:DQK].reshape(2 * ND, T).T),
        "gelub": np.ascontiguousarray(gelub_eff.reshape(NF, T).T),
        "obrep": np.ascontiguousarray(
            np.broadcast_to(ob_eff[None, :], (T, D))).astype(bf),
        "b2rep": np.ascontiguousarray(
            np.broadcast_to(b2row_eff[None, :], (T, D))).astype(bf),
        "g1rep": np.ascontiguousarray(
            np.broadcast_to(np.asarray(n1_g, f32)[None, :], (T, D))).astype(bf),
        "g2rep": np.ascontiguousarray(
            np.broadcast_to(np.asarray(n2_g, f32)[None, :], (T, D))).astype(bf),
        "n2brep": np.ascontiguousarray(
            np.broadcast_to(np.asarray(n2_b, f32)[None, :], (T, D))),
    }

    # masks in [k, (h4, kb, q)] layout, 0/1 bf16, tiled over 4 heads
    k_i = np.arange(T, dtype=np.int64)[:, None]
    q_i = np.arange(T, dtype=np.int64)[None, :]
    m_kb0 = (k_i > q_i).astype(f32)         # previous key block
    m_kb1 = (k_i <= q_i).astype(f32)        # current key block (causal)
    rest = np.concatenate([m_kb0, m_kb1], axis=1)          # [T, 2T]
    first = np.concatenate([np.zeros((T, T), f32), m_kb1], axis=1)
    m01r = np.ascontiguousarray(np.tile(rest, (1, 4))).astype(bf)
    m01f_bs = np.ascontiguousarray(np.tile(first, (1, 4))).astype(bf)

    in_maps = []
    for c in range(N_CORES):
        b, i0 = divmod(c * NTOK, S)
        halo = (np.zeros((T, D), f32) if i0 == 0 else x[b, i0 - T:i0])
        xh = np.ascontiguousarray(
            np.concatenate([halo, x[b, i0:i0 + NTOK]], axis=0)).astype(bf)
        m = dict(shared)
        m["xh"] = xh
        m["m01f"] = m01f_bs if i0 == 0 else m01r
        m["m01r"] = m01r
        in_maps.append(m)
    return in_maps


def kernel(**inputs):
    from concourse.bass_utils import run_bass_kernel_spmd
    nc = _get_nc()
    in_maps = make_in_maps(**inputs)
    res = run_bass_kernel_spmd(nc, in_maps, core_ids=list(range(N_CORES)))
    outs = [res.results[c]["out"] for c in range(N_CORES)]
    return np.concatenate(outs, axis=0).reshape(B, S, D).astype(np.float32)
